# revision 16
# baseline (speedup 1.0000x reference)
"""Trainium2 Bass kernel for nn_Block_72138270704025 (windowed sparse attention
block: LN1 -> window partition -> MHA with decomposed rel-pos bias gathered by
q_idx/k_idx -> window unpartition -> residual -> LN2 -> MLP(gelu) -> residual).

Sharding: data-parallel over batch B=8, one batch element per NeuronCore; all
weights replicated.  Host folds LN affine params into the adjacent matmul
weights, precomputes the rel-pos tables Sh/Sw, and turns the per-(window,head)
index gathers into two small (28 x 196) operands per attention batch so the
bias folds into the logits matmul as a PSUM-accumulated matmul.

v2 optimizations over the bf16 baseline:
- fp8e4m3 DoubleRow matmuls (2 k-tiles per pass) for qkv/v/proj/fc1 and the
  PV+denominator matmuls (weights scaled x32 on host to clear the e4m3
  subnormal band; descale folded into the existing PSUM-evacuation ops).
- Softmax denominator reciprocal via the custom-DVE reciprocal_approx_fast
  (~5x cheaper than the iterative-divide microcode), one per head-PAIR: even
  head lands in PSUM rows 0:64, odd head rows 64:128 of one bank, so a single
  reciprocal + multiply normalizes both heads.
- Head-software-pipelining: QK/bias matmuls of head pair j+1 are emitted
  before the PV matmuls of pair j so the PE never stalls on the ACT-engine
  exp; keeps the HAM clock-gate at K=8/8 (the bf16 baseline ran most
  attention matmuls at half clock).
- Transposes as regular identity matmuls (counts as PE activity for HAM and
  is ~2.5x cheaper than transpose-mode).
- LN sqrt batched per band/group (one ACT sqrt per 7 tiles) to cut act-table
  set switches; rsqrt finished with reciprocal_approx_fast on DVE.
- Phase C (residual+LN2+MLP) groups interleaved into the band loop as their
  token ranges complete, removing the phase boundary bubble.
- Window gather/scatter as composite 3D DMAs (2 per window instead of 14)
  for interior windows; attn intermediate stored bf16.
"""
import os
import sys

for _p in ('/opt/trn_rl_repo', '/root/.axon_site/_ro/trn_rl_repo'):
    if os.path.isdir(_p) and _p not in sys.path:
        sys.path.append(_p)

import numpy as np
import ml_dtypes

import concourse.bass as bass
import concourse.tile as tile
from concourse import mybir
from concourse.bass_utils import run_bass_kernel_spmd
from concourse.tile import ScopedClock
from concourse.masks import make_identity

# ---- problem constants (hardcoded per contest rules) ----
B = 8
HH = 64
WW = 64
DIM = 768
NH = 12
WS = 14
HD = 64
N = 196            # tokens per window
NWS = 5            # window grid side
NW = 25            # windows per image
EPS = 1e-5
NTOK = HH * WW     # 4096 tokens per core
CH = 98            # window token chunk: 7 rows of 14 (196 = 2x98)
WSCALE = 32.0      # host fp8 weight scale (descaled at PSUM evacuation)

F32 = mybir.dt.float32
BF16 = mybir.dt.bfloat16
F8 = mybir.dt.float8e4
DR = mybir.MatmulPerfMode.DoubleRow


def _patch_tile_drain():
    """Walrus CoreV3 codegen rejects a Drain carrying multiple sem waits
    ("Too many sync wait commands").  Emit explicit wait_ge instructions
    before the kernel-tail drain instead."""
    if getattr(tile.TileContext, '_drain_patched', False):
        return

    def _drain_and_barrier(self, tick_clock, wait_clock):
        nc = self.nc
        dummy = nc.sync.nop(nofuse=True)
        wait_clock.add_sem_waits(dummy.ins, ScopedClock({None: tick_clock.global_clock}))
        waits = list(dummy.ins.sync_info.on_wait or [])
        dummy.ins.sync_info.on_wait = []
        assert self.sems is not None
        by_id = {}
        for h in self.sems.allocated().values():
            by_id[getattr(h, 'id', None)] = h
            by_id[getattr(h, 'name', None)] = h
        for w in waits:
            h = by_id.get(w.id) or by_id.get(w.ant_name)
            assert h is not None, (w.id, w.ant_name)
            nc.sync.wait_ge(h, w.wait_value)
        nc.sync.drain()
        nc.all_engine_barrier()
        popped = nc._tile_sem_poison_stack.pop()
        assert popped is self._sem_poison
        nc.clear_and_free_semaphores(list(self.sems.allocated().values()))
        nc.all_engine_barrier()

    tile.TileContext._drain_and_barrier = _drain_and_barrier
    tile.TileContext._drain_patched = True


def _install_ntff_hook():
    """Recreate the missing antenv.axon_hooks module so trace=True can reach
    the axon NTFF profiler (used only when KERNEL_TRACE=1)."""
    try:
        import types
        import antenv
        if 'antenv.axon_hooks' in sys.modules:
            return True
        mod = types.ModuleType('antenv.axon_hooks')
        mod._hook = None
        mod.set_axon_ntff_profile_hook = lambda h: setattr(mod, '_hook', h)
        mod.get_axon_ntff_profile_hook = lambda: mod._hook
        sys.modules['antenv.axon_hooks'] = mod
        antenv.axon_hooks = mod
        from trn_agent_boot.trn_boot import _ntff_profile_via_ctypes
        mod._hook = _ntff_profile_via_ctypes('/opt/axon/libaxon_pjrt.so')
        return mod._hook is not None
    except Exception:
        return False


# window geometry helpers
def _win_rc(w):
    return w // NWS, w % NWS


def _valid(w):
    wr, wc = _win_rc(w)
    return (14 if wr < 4 else 8), (14 if wc < 4 else 8)


_CACHE = {}


def _dedup_ldweights(nc):
    """Tile lowers each matmul to Ldweights+Matmult.  Back-to-back matmuls
    that share a stationary operand reload identical weights; drop the
    redundant Ldweights (keeping its sem waits / updates on a zero-cost
    EventSemaphore)."""
    ndrop = 0
    for fn in nc.m.functions:
        for blk in fn.blocks:
            insts = blk.instructions
            out = []
            prev_key = None
            dirty = False
            for ins in insts:
                if ins.engine != mybir.EngineType.PE:
                    out.append(ins)
                    continue
                if ins.opcode == 'Ldweights':
                    a = ins.ins[0]
                    key = (str(getattr(a, 'memory_location', None)),
                           getattr(a, 'offset', None), str(getattr(a, 'ap', None)),
                           str(getattr(ins, 'is_transpose', None)),
                           str(getattr(ins, 'perf_mode', None)))
                    si = ins.sync_info
                    has_sync = si and (si.on_wait or si.on_update)
                    if key == prev_key:
                        ndrop += 1
                        dirty = True
                        if has_sync:
                            ev = mybir.InstEventSemaphore(
                                name=f"LDDROP-{nc.next_id()}", ins=[], outs=[])
                            ev.engine = ins.engine
                            ev.sync_info = mybir.SyncInfo(
                                on_wait=list(si.on_wait or []),
                                on_update=list(si.on_update or []))
                            out.append(ev)
                        continue
                    prev_key = key
                    out.append(ins)
                elif ins.opcode == 'Matmult' and not getattr(ins, 'is_transpose', False):
                    out.append(ins)
                else:
                    prev_key = None
                    out.append(ins)
            if dirty:
                blk.instructions = out
    return ndrop


def _split_waits(nc, cap=None):
    """Walrus CoreV2/V3 codegen rejects instructions whose sync_info carries
    more waits than the per-opcode ISA ctrl struct holds.  Hoist excess waits
    onto standalone EventSemaphore instructions."""
    if cap is None:
        cap = int(os.environ.get('KERNEL_MAXWAITS', '1'))
    n_split = 0
    for fn in nc.m.functions:
        for blk in fn.blocks:
            insts = blk.instructions
            out = []
            dirty = False
            for ins in insts:
                si = ins.sync_info
                waits = list(si.on_wait) if si and si.on_wait else []
                limit = 1 if ins.opcode in ('Drain',) else cap
                if len(waits) > limit:
                    keep, extra = waits[:limit], waits[limit:]
                    for k in range(0, len(extra), cap):
                        ev = mybir.InstEventSemaphore(
                            name=f"WSPLIT-{nc.next_id()}", ins=[], outs=[])
                        ev.engine = ins.engine
                        ev.sync_info = mybir.SyncInfo(
                            on_wait=extra[k:k + cap], on_update=[])
                        out.append(ev)
                        n_split += 1
                    si.on_wait = keep
                    dirty = True
                out.append(ins)
            if dirty:
                blk.instructions = out
    return n_split


def _build():
    if 'nc' in _CACHE:
        return _CACHE['nc']
    _patch_tile_drain()

    nc = bass.Bass()

    # ---- dram parameters ----
    x_d = nc.dram_tensor("x", [NTOK, DIM], F32, kind="ExternalInput")
    eq_d = nc.dram_tensor("eq", [NW, NH, 28, N], F8, kind="ExternalInput")
    ek_d = nc.dram_tensor("ek", [NW, NH, 28, N], F8, kind="ExternalInput")
    wqk_d = nc.dram_tensor("wqk", [DIM, 2 * DIM], F8, kind="ExternalInput")
    wv_d = nc.dram_tensor("wv", [DIM, DIM], F8, kind="ExternalInput")
    bqk_d = nc.dram_tensor("bqk", [12, 128], F32, kind="ExternalInput")
    vb_d = nc.dram_tensor("vb", [1, DIM], F32, kind="ExternalInput")
    wp_d = nc.dram_tensor("wp", [DIM, DIM], F8, kind="ExternalInput")
    pb_d = nc.dram_tensor("pb", [1, DIM], F32, kind="ExternalInput")
    w1_d = nc.dram_tensor("w1", [DIM, 4 * DIM], F8, kind="ExternalInput")
    b1_d = nc.dram_tensor("b1", [24, 128], F32, kind="ExternalInput")
    w2_d = nc.dram_tensor("w2", [4 * DIM, DIM], BF16, kind="ExternalInput")
    b2_d = nc.dram_tensor("b2", [1, DIM], F32, kind="ExternalInput")
    y_d = nc.dram_tensor("y", [NTOK, DIM], F32, kind="ExternalOutput")

    dbg = os.environ.get('KERNEL_DEBUG') == '1'
    skind = dict(kind="ExternalOutput") if dbg else {}
    # xn1 banded by window row (7/7/7/7/4 token tiles)
    band_tiles = [7, 7, 7, 7, 4]
    xn1_b = [nc.dram_tensor(f"xn1b{i}", [band_tiles[i] * 128, DIM], F8)
             for i in range(5)]
    at_d = nc.dram_tensor("attn", [NTOK, DIM], BF16, **skind)

    x_t32 = x_d.rearrange("(a p) d -> a p d", p=128)      # 32 token tiles
    x_pt = x_d.rearrange("(a p) d -> p a d", p=128)       # grouped loads
    xn1b_t = [t.rearrange("(a p) d -> a p d", p=128) for t in xn1_b]
    xn1b_img = [t.rearrange("(r c) d -> r c d", c=WW) for t in xn1_b]
    at_img = at_d.rearrange("(r c) d -> r c d", c=WW)
    at_pt = at_d.rearrange("(a p) d -> p a d", p=128)
    y_t32 = y_d.rearrange("(a p) d -> a p d", p=128)

    inv_w = 1.0 / WSCALE

    with tile.TileContext(nc, pool_alloc_mode='queue') as tc:
        with tc.tile_pool(name="cW", bufs=1) as pcw, \
             tc.tile_pool(name="lnA", bufs=2) as pa, \
             tc.tile_pool(name="xtP", bufs=8) as pxt, \
             tc.tile_pool(name="xwP", bufs=2) as pxw, \
             tc.tile_pool(name="xwtP", bufs=2) as pxwt, \
             tc.tile_pool(name="qkP", bufs=2) as pqk, \
             tc.tile_pool(name="eqP", bufs=1) as peq, \
             tc.tile_pool(name="vP", bufs=2) as pv, \
             tc.tile_pool(name="hdP", bufs=4) as phd, \
             tc.tile_pool(name="owP", bufs=2) as pow_, \
             tc.tile_pool(name="gC", bufs=2) as pg, \
             tc.tile_pool(name="agC", bufs=1) as pag, \
             tc.tile_pool(name="yC", bufs=2) as py, \
             tc.tile_pool(name="hC", bufs=1) as ph, \
             tc.tile_pool(name="gX", bufs=2) as pgx, \
             tc.tile_pool(name="psB", bufs=6, space="PSUM") as psb, \
             tc.tile_pool(name="ptB", bufs=2, space="PSUM") as ptb:

            # ---- persistent weights / consts ----
            w1_sb = pcw.tile([128, 6, 4 * DIM], F8)
            nc.sync.dma_start(out=w1_sb[:], in_=w1_d.rearrange("(k p) n -> p k n", p=128))
            b1_sb = pcw.tile([128, 24], F32)
            nc.sync.dma_start(out=b1_sb[:], in_=b1_d.rearrange("a p -> p a"))
            if not _CACHE.get('b2_zero'):
                b2_sb = pcw.tile([128, DIM], F32)
                nc.gpsimd.dma_start(out=b2_sb[:], in_=b2_d[0:1, :].to_broadcast((128, DIM)))
            w2_sb = pcw.tile([128, 24, DIM], BF16)
            nc.sync.dma_start(out=w2_sb[:], in_=w2_d.rearrange("(k p) n -> p k n", p=128))
            eps_t = pcw.tile([128, 1], F32)
            nc.vector.memset(eps_t[:], EPS)
            ident = pcw.tile([128, 128], F8)
            make_identity(nc, ident[:])
            wqk_sb = pcw.tile([128, 6, 2 * DIM], F8)
            nc.sync.dma_start(out=wqk_sb[:], in_=wqk_d.rearrange("(k p) n -> p k n", p=128))
            wv_sb = pcw.tile([128, 6, DIM], F8)
            nc.sync.dma_start(out=wv_sb[:], in_=wv_d.rearrange("(k p) n -> p k n", p=128))
            wp_sb = pcw.tile([128, 6, DIM], F8)
            nc.sync.dma_start(out=wp_sb[:], in_=wp_d.rearrange("(k p) n -> p k n", p=128))
            bqk_sb = pcw.tile([128, 12], F32)
            nc.sync.dma_start(out=bqk_sb[:], in_=bqk_d.rearrange("a p -> p a"))
            if not _CACHE.get('vb_zero'):
                vb_sb = pcw.tile([128, DIM], F32)
                nc.gpsimd.dma_start(out=vb_sb[:], in_=vb_d[0:1, :].to_broadcast((128, DIM)))
            if not _CACHE.get('pb_zero'):
                pb_sb = pcw.tile([128, DIM], F32)
                nc.gpsimd.dma_start(out=pb_sb[:], in_=pb_d[0:1, :].to_broadcast((128, DIM)))

            sig_gelu = os.environ.get('KERNEL_GELU') == 'sig'

            def emit_ln1_band(band):
                """LN1 for this band's token tiles; batched sqrt for the band."""
                nbt = band_tiles[band]
                xts = []
                mvb = pa.tile([128, 2, 7], F32, tag="mvb")
                for bt in range(nbt):
                    t = band * 7 + bt
                    xt = pxt.tile([128, DIM], F32, tag="xt")
                    nc.sync.dma_start(out=xt[:], in_=x_t32[t])
                    st = pa.tile([128, 2, 6], F32, tag="st")
                    for s in range(2):
                        nc.vector.bn_stats(out=st[:, s, :], in_=xt[:, s * 384:(s + 1) * 384])
                    nc.vector.bn_aggr(out=mvb[:, :, bt], in_=st[:])
                    xts.append(xt)
                sdb = pa.tile([128, 7], F32, tag="sdb")
                nc.scalar.activation(out=sdb[:, 0:nbt], in_=mvb[:, 1, 0:nbt],
                                     func=mybir.ActivationFunctionType.Sqrt,
                                     bias=eps_t[:], scale=1.0)
                rsd = pa.tile([128, 7], F32, tag="rsd")
                nc.vector.reciprocal(out=rsd[:, 0:nbt], in_=sdb[:, 0:nbt])
                for bt in range(nbt):
                    xn = pa.tile([128, DIM], F8, tag="xn")
                    nc.vector.tensor_scalar(out=xn[:], in0=xts[bt][:],
                                            scalar1=mvb[:, 0, bt:bt + 1],
                                            scalar2=rsd[:, bt:bt + 1],
                                            op0=mybir.AluOpType.subtract,
                                            op1=mybir.AluOpType.mult)
                    nc.sync.dma_start(out=xn1b_t[band][bt], in_=xn[:])

            def emit_window_group(wins):
                """One group (pair or lone window): qkv, per-window V + pipelined
                heads + proj + scatter."""
                nwin = len(wins)
                wfree = N * nwin
                FPAD = 400 if nwin == 2 else 208   # fp8 Ko-step must be %16
                xwtb = pxwt.tile([128, 6, FPAD], F8, tag="xwtb")
                qkt = pqk.tile([128, 12, wfree], F8, tag="qkt")
                att = pxwt.tile([128, 6, FPAD], F8, tag="att")

                # gather + transpose into xwtb
                for ww_i, w in enumerate(wins):
                    woff = ww_i * N
                    wr, wc = _win_rc(w)
                    vr, vc = _valid(w)
                    edge = (vr < 14) or (vc < 14)
                    xw = pxw.tile([128, 2, DIM], F8, tag="xw")
                    if edge:
                        nc.gpsimd.memset(xw[0:CH, 0, :], 0.0)
                        nc.gpsimd.memset(xw[0:CH, 1, :], 0.0)
                        for r in range(vr):
                            c, p0 = r // 7, (r % 7) * WS
                            nc.gpsimd.dma_start(
                                out=xw[p0:p0 + vc, c, :],
                                in_=xn1b_img[wr][r, wc * WS:wc * WS + vc, :])
                    else:
                        for c in range(2):
                            nc.gpsimd.dma_start(
                                out=xw[0:CH, c, :],
                                in_=xn1b_img[wr][c * 7:c * 7 + 7,
                                                 wc * WS:wc * WS + WS, :])
                    # transpose via regular identity matmul: out = xw_slice.T @ I
                    for c, coff in ((0, 0), (1, CH)):
                        for j in range(6):
                            pt = ptb.tile([128, 128], F32, tag="pt")
                            nc.tensor.matmul(
                                pt[0:128, 0:CH],
                                lhsT=xw[0:CH, c, j * 128:(j + 1) * 128],
                                rhs=ident[0:CH, 0:CH],
                                start=True, stop=True)
                            nc.vector.tensor_copy(
                                out=xwtb[:, j, woff + coff:woff + coff + CH],
                                in_=pt[0:128, 0:CH])

                # qkv^T for the whole group (fp8 DoubleRow over k-tile pairs)
                for oc in range(12):
                    pqm = psb.tile([128, 392], F32, tag="ps")
                    for kp in range(3):
                        nc.tensor.matmul(
                            pqm[:, 0:wfree],
                            lhsT=wqk_sb[:, 2 * kp:2 * kp + 2, oc * 128:(oc + 1) * 128],
                            rhs=xwtb[:, 2 * kp:2 * kp + 2, 0:wfree],
                            perf_mode=DR,
                            start=(kp == 0), stop=(kp == 2))
                    nc.vector.tensor_scalar(out=qkt[:, oc, :], in0=pqm[:, 0:wfree],
                                            scalar1=inv_w,
                                            scalar2=bqk_sb[:, oc:oc + 1],
                                            op0=mybir.AluOpType.mult,
                                            op1=mybir.AluOpType.add)

                for ww_i, w in enumerate(wins):
                    woff = ww_i * N
                    # V (fp8): all heads + 64 ones columns for the denominator
                    va = pv.tile([128, 2, DIM + 64], F8, tag="va")
                    for c, coff in ((0, 0), (1, CH)):
                        nc.gpsimd.memset(va[0:CH, c, DIM:DIM + 64], 1.0)
                        pv0 = psb.tile([128, 384], F32, tag="ps")
                        pv1 = psb.tile([128, 384], F32, tag="ps")
                        for kp in range(3):
                            nc.tensor.matmul(
                                pv0[0:CH, :],
                                lhsT=xwtb[:, 2 * kp:2 * kp + 2,
                                          woff + coff:woff + coff + CH],
                                rhs=wv_sb[:, 2 * kp:2 * kp + 2, 0:384],
                                perf_mode=DR,
                                start=(kp == 0), stop=(kp == 2))
                            nc.tensor.matmul(
                                pv1[0:CH, :],
                                lhsT=xwtb[:, 2 * kp:2 * kp + 2,
                                          woff + coff:woff + coff + CH],
                                rhs=wv_sb[:, 2 * kp:2 * kp + 2, 384:768],
                                perf_mode=DR,
                                start=(kp == 0), stop=(kp == 2))
                        for half, pvm in ((0, pv0), (1, pv1)):
                            if _CACHE.get('vb_zero'):
                                nc.vector.tensor_scalar(
                                    out=va[0:CH, c, half * 384:(half + 1) * 384],
                                    in0=pvm[0:CH, :], scalar1=inv_w, scalar2=None,
                                    op0=mybir.AluOpType.mult)
                            else:
                                nc.vector.scalar_tensor_tensor(
                                    out=va[0:CH, c, half * 384:(half + 1) * 384],
                                    in0=pvm[0:CH, :],
                                    scalar=inv_w,
                                    in1=vb_sb[0:CH, half * 384:(half + 1) * 384],
                                    op0=mybir.AluOpType.mult,
                                    op1=mybir.AluOpType.add)

                    eqt = peq.tile([28, NH, N], F8, tag="eqt")
                    nc.sync.dma_start(out=eqt[:], in_=eq_d[w].rearrange("h r i -> r h i"))
                    ekt = peq.tile([28, NH, N], F8, tag="ekt")
                    nc.sync.dma_start(out=ekt[:], in_=ek_d[w].rearrange("h r i -> r h i"))

                    # heads: QK/bias+exp for pair p, then PV/normalize for pair p-1
                    pTs = {}
                    psos = {}

                    def emit_qk(h):
                        hp = (h % 2) * 64
                        qT = qkt[hp:hp + 64, h // 2, woff:woff + N]
                        kT = qkt[hp:hp + 64, 6 + h // 2, woff:woff + N]
                        pss = psb.tile([128, 2 * N], F32, tag="ps")
                        for c in range(2):
                            nc.tensor.matmul(pss[0:CH, c * N:(c + 1) * N],
                                             lhsT=kT[:, c * CH:(c + 1) * CH], rhs=qT,
                                             start=True, stop=False)
                            nc.tensor.matmul(pss[0:CH, c * N:(c + 1) * N],
                                             lhsT=ekt[:, h, c * CH:(c + 1) * CH],
                                             rhs=eqt[:, h, :],
                                             start=False, stop=True)
                        pT = phd.tile([128, 2, 208], F8, tag="pT")
                        nc.scalar.activation(out=pT[0:CH, :, 0:N], in_=pss[0:CH, 0:2 * N],
                                             func=mybir.ActivationFunctionType.Exp)
                        pTs[h] = pT

                    def emit_pv(p):
                        pso = psb.tile([128, 500], F32, tag="ps")
                        # HAM-warming dummies: keep the PE array active while
                        # PV waits on the ACT exp; scratch region 392:492 of
                        # the same bank, never read.
                        for dmy in range(8):
                            nc.tensor.matmul(
                                pso[0:64, 392:492],
                                lhsT=ident[0:64, (dmy % 2) * 64:(dmy % 2) * 64 + 64],
                                rhs=ident[0:64, 0:100],
                                start=True, stop=True, skip_group_check=True)
                        for h in (2 * p, 2 * p + 1):
                            b0 = (h % 2) * 64
                            pT = pTs.pop(h)
                            if b0 == 0:
                                # DoubleRow folds both key-chunks into one pass
                                nc.tensor.matmul(pso[0:64, 0:N],
                                                 lhsT=va[0:CH, 0:2, h * 64:(h + 1) * 64],
                                                 rhs=pT[0:CH, 0:2, 0:N],
                                                 perf_mode=DR, start=True, stop=True,
                                                 skip_group_check=True)
                                nc.tensor.matmul(pso[0:64, N:2 * N],
                                                 lhsT=va[0:CH, 0:2, DIM:DIM + 64],
                                                 rhs=pT[0:CH, 0:2, 0:N],
                                                 perf_mode=DR, start=True, stop=True,
                                                 skip_group_check=True)
                            else:
                                # walrus rejects DoubleRow + col-offset
                                # tile_position; plain fp8 per chunk instead
                                for c in range(2):
                                    nc.tensor.matmul(pso[64:128, 0:N],
                                                     lhsT=va[0:CH, c, h * 64:(h + 1) * 64],
                                                     rhs=pT[0:CH, c, 0:N],
                                                     start=(c == 0), stop=(c == 1),
                                                     skip_group_check=True)
                                    nc.tensor.matmul(pso[64:128, N:2 * N],
                                                     lhsT=va[0:CH, c, DIM:DIM + 64],
                                                     rhs=pT[0:CH, c, 0:N],
                                                     start=(c == 0), stop=(c == 1),
                                                     skip_group_check=True)
                        psos[p] = pso

                    def emit_norm(p):
                        pso = psos.pop(p)
                        rb = phd.tile([128, N], F32, tag="rb")
                        nc.vector.reciprocal(out=rb[:], in_=pso[:, N:2 * N])
                        nc.vector.tensor_mul(out=att[:, p, woff:woff + N],
                                             in0=pso[:, 0:N], in1=rb[:])

                    for p in range(6):
                        emit_qk(2 * p)
                        emit_qk(2 * p + 1)
                        if p >= 1:
                            emit_pv(p - 1)
                            emit_norm(p - 1)
                    emit_pv(5)
                    emit_norm(5)

                    # proj (fp8 DoubleRow) -> ow, then unpartition scatter
                    ow = pow_.tile([128, 2, DIM], BF16, tag="ow")
                    for c, coff in ((0, 0), (1, CH)):
                        pp0 = psb.tile([128, 384], F32, tag="ps")
                        pp1 = psb.tile([128, 384], F32, tag="ps")
                        for kp in range(3):
                            nc.tensor.matmul(
                                pp0[0:CH, :],
                                lhsT=att[:, 2 * kp:2 * kp + 2,
                                         woff + coff:woff + coff + CH],
                                rhs=wp_sb[:, 2 * kp:2 * kp + 2, 0:384],
                                perf_mode=DR,
                                start=(kp == 0), stop=(kp == 2))
                            nc.tensor.matmul(
                                pp1[0:CH, :],
                                lhsT=att[:, 2 * kp:2 * kp + 2,
                                         woff + coff:woff + coff + CH],
                                rhs=wp_sb[:, 2 * kp:2 * kp + 2, 384:768],
                                perf_mode=DR,
                                start=(kp == 0), stop=(kp == 2))
                        for half, psp in ((0, pp0), (1, pp1)):
                            if _CACHE.get('pb_zero'):
                                nc.scalar.activation(
                                    out=ow[0:CH, c, half * 384:(half + 1) * 384],
                                    in_=psp[0:CH, :],
                                    func=mybir.ActivationFunctionType.Copy,
                                    bias=0.0, scale=inv_w)
                            else:
                                nc.vector.scalar_tensor_tensor(
                                    out=ow[0:CH, c, half * 384:(half + 1) * 384],
                                    in0=psp[0:CH, :], scalar=inv_w,
                                    in1=pb_sb[0:CH, half * 384:(half + 1) * 384],
                                    op0=mybir.AluOpType.mult,
                                    op1=mybir.AluOpType.add)
                    wr, wc = _win_rc(w)
                    vr, vc = _valid(w)
                    if vr == 14 and vc == 14:
                        for c in range(2):
                            nc.gpsimd.dma_start(
                                out=at_img[wr * WS + c * 7:wr * WS + c * 7 + 7,
                                           wc * WS:wc * WS + WS, :],
                                in_=ow[0:CH, c, :])
                    else:
                        for r in range(vr):
                            c, p0 = r // 7, (r % 7) * WS
                            nc.gpsimd.dma_start(
                                out=at_img[wr * WS + r, wc * WS:wc * WS + vc, :],
                                in_=ow[p0:p0 + vc, c, :])

            def emit_c_group(g):
                """Phase C for token tiles 2g..2g+1: residual, LN2, MLP, out."""
                xg = pg.tile([128, 2, DIM], F32, tag="xg")
                ag = pag.tile([128, 2, DIM], BF16, tag="ag")
                nc.sync.dma_start(out=xg[:], in_=x_pt[:, 2 * g:2 * g + 2, :])
                nc.sync.dma_start(out=ag[:], in_=at_pt[:, 2 * g:2 * g + 2, :])
                # x2 = x + attn (in place into xg)
                nc.vector.tensor_add(out=xg[:, :, :], in0=xg[:, :, :], in1=ag[:, :, :])
                xn2t = pgx.tile([128, 6, 256], F8, tag="xn2t")
                mvc = pg.tile([128, 2, 2], F32, tag="mvc")
                for s in range(2):
                    st = pg.tile([128, 2, 6], F32, tag="stC")
                    for sub in range(2):
                        nc.vector.bn_stats(out=st[:, sub, :],
                                           in_=xg[:, s, sub * 384:(sub + 1) * 384])
                    nc.vector.bn_aggr(out=mvc[:, :, s], in_=st[:])
                sdc = pg.tile([128, 2], F32, tag="sdC")
                nc.scalar.activation(out=sdc[:], in_=mvc[:, 1, :],
                                     func=mybir.ActivationFunctionType.Sqrt,
                                     bias=eps_t[:], scale=1.0)
                rsc = pg.tile([128, 2], F32, tag="rsC")
                nc.vector.reciprocal(out=rsc[:], in_=sdc[:])
                for s in range(2):
                    xn2b = pg.tile([128, DIM], F8, tag="xn2b")
                    nc.vector.tensor_scalar(out=xn2b[:, :], in0=xg[:, s, :],
                                            scalar1=mvc[:, 0, s:s + 1],
                                            scalar2=rsc[:, s:s + 1],
                                            op0=mybir.AluOpType.subtract,
                                            op1=mybir.AluOpType.mult)
                    if not _CACHE.get('b2_zero'):
                        nc.vector.tensor_add(out=xg[:, s, :], in0=xg[:, s, :],
                                             in1=b2_sb[:])
                    for j in range(6):
                        pt = ptb.tile([128, 128], F32, tag="pt")
                        nc.tensor.matmul(pt[:, :],
                                         lhsT=xn2b[:, j * 128:(j + 1) * 128],
                                         rhs=ident[:, :], start=True, stop=True)
                        nc.vector.tensor_copy(out=xn2t[:, j, s * 128:(s + 1) * 128],
                                              in_=pt[:, :])
                h1t = ph.tile([128, 24, 256], BF16, tag="h1t")
                for oc in range(24):
                    psh = psb.tile([128, 256], F32, tag="ps")
                    for kp in range(3):
                        nc.tensor.matmul(
                            psh[:, :],
                            lhsT=w1_sb[:, 2 * kp:2 * kp + 2, oc * 128:(oc + 1) * 128],
                            rhs=xn2t[:, 2 * kp:2 * kp + 2, :],
                            perf_mode=DR,
                            start=(kp == 0), stop=(kp == 2))
                    if sig_gelu:
                        # CoreSim lacks Gelu; x*sigmoid(1.702x) validates shapes
                        hpre = pg.tile([128, 256], F32, tag="hpre")
                        nc.scalar.activation(out=hpre[:], in_=psh[:, :],
                                             func=mybir.ActivationFunctionType.Identity,
                                             bias=b1_sb[:, oc:oc + 1], scale=inv_w)
                        sg = pg.tile([128, 256], F32, tag="sg")
                        nc.scalar.activation(out=sg[:], in_=hpre[:],
                                             func=mybir.ActivationFunctionType.Sigmoid,
                                             bias=0.0, scale=1.702)
                        nc.vector.tensor_mul(out=h1t[:, oc, :], in0=hpre[:], in1=sg[:])
                    else:
                        nc.scalar.activation(out=h1t[:, oc, :], in_=psh[:, :],
                                             func=mybir.ActivationFunctionType.Gelu,
                                             bias=b1_sb[:, oc:oc + 1], scale=inv_w)
                for s in range(2):
                    pf0 = psb.tile([128, 384], F32, tag="ps")
                    pf1 = psb.tile([128, 384], F32, tag="ps")
                    for kt in range(24):
                        nc.tensor.matmul(
                            pf0[:, :],
                            lhsT=h1t[:, kt, s * 128:(s + 1) * 128],
                            rhs=w2_sb[:, kt, 0:384],
                            start=(kt == 0), stop=(kt == 23))
                        nc.tensor.matmul(
                            pf1[:, :],
                            lhsT=h1t[:, kt, s * 128:(s + 1) * 128],
                            rhs=w2_sb[:, kt, 384:768],
                            start=(kt == 0), stop=(kt == 23))
                    yo = py.tile([128, DIM], F32, tag="yo")
                    for half, psf in ((0, pf0), (1, pf1)):
                        nc.vector.tensor_add(
                            out=yo[:, half * 384:(half + 1) * 384],
                            in0=psf[:, :],
                            in1=xg[:, s, half * 384:(half + 1) * 384])
                    nc.sync.dma_start(out=y_t32[2 * g + s], in_=yo[:])

            # groups of phase C that become ready after each band completes:
            # band b covers image rows 14b..14b+13 -> tokens to (14b+14)*64
            c_ready = {0: [0, 1, 2], 1: [3, 4, 5, 6], 2: [7, 8, 9],
                       3: [10, 11, 12, 13], 4: [14, 15]}

            emit_ln1_band(0)
            for band in range(5):
                w0 = band * NWS
                emit_window_group((w0, w0 + 1))
                # overlap next band's LN1 (DVE/DMA) with this band's windows
                if band < 4:
                    emit_ln1_band(band + 1)
                emit_window_group((w0 + 2, w0 + 3))
                emit_window_group((w0 + 4,))
                for g in c_ready[band]:
                    emit_c_group(g)

    if os.environ.get('KERNEL_NOLDDEDUP') != '1':
        _dedup_ldweights(nc)
    if os.environ.get('KERNEL_SIM') != '1':
        _split_waits(nc)
    _CACHE['nc'] = nc
    return nc


def _host_prep(inputs):
    """Fold LN affines into matmul weights, build rel-pos operands."""
    f32 = np.float32
    x = np.asarray(inputs['x'], f32)
    q_idx = np.asarray(inputs['q_idx']).astype(np.int64)
    k_idx = np.asarray(inputs['k_idx']).astype(np.int64)
    ln1_w = np.asarray(inputs['ln1_w'], f32); ln1_b = np.asarray(inputs['ln1_b'], f32)
    ln2_w = np.asarray(inputs['ln2_w'], f32); ln2_b = np.asarray(inputs['ln2_b'], f32)
    qkv_w = np.asarray(inputs['qkv_w'], f32); qkv_b = np.asarray(inputs['qkv_b'], f32)
    proj_w = np.asarray(inputs['proj_w'], f32); proj_b = np.asarray(inputs['proj_b'], f32)
    mlp_w1 = np.asarray(inputs['mlp_w1'], f32); mlp_b1 = np.asarray(inputs['mlp_b1'], f32)
    mlp_w2 = np.asarray(inputs['mlp_w2'], f32); mlp_b2 = np.asarray(inputs['mlp_b2'], f32)
    rel_h = np.asarray(inputs['rel_h'], f32); rel_w = np.asarray(inputs['rel_w'], f32)

    scale = HD ** -0.5
    Wqkv = ln1_w[:, None] * qkv_w
    bqkv = ln1_b @ qkv_w + qkv_b
    Wqkv = Wqkv.copy(); bqkv = bqkv.copy()
    Wqkv[:, :DIM] *= scale
    bqkv[:DIM] *= scale
    W1 = ln2_w[:, None] * mlp_w1
    b1 = ln2_b @ mlp_w1 + mlp_b1

    coords = np.arange(WS)[:, None] - np.arange(WS)[None, :] + (WS - 1)
    Sh = rel_h[coords].sum(-1).astype(f32)
    Sw = rel_w[coords].sum(-1).astype(f32)

    qr, qc = q_idx // WS, q_idx % WS
    kr, kc = k_idx // WS, k_idx % WS
    nb = q_idx.shape[0]
    Eq = np.concatenate([np.take(Sh, qr, axis=0).transpose(0, 2, 1),
                         np.take(Sw, qc, axis=0).transpose(0, 2, 1)], axis=1)
    Ek = np.zeros((nb, 28, N), f32)
    bi = np.arange(nb)[:, None]
    ar = np.arange(N)[None, :]
    Ek[bi, kr, ar] = 1.0
    Ek[bi, WS + kc, ar] = 1.0

    bf = ml_dtypes.bfloat16
    f8 = ml_dtypes.float8_e4m3fn
    shared = {
        "wqk": np.ascontiguousarray(Wqkv[:, :2 * DIM] * WSCALE).astype(f8),
        "wv": np.ascontiguousarray(Wqkv[:, 2 * DIM:] * WSCALE).astype(f8),
        "bqk": np.ascontiguousarray(bqkv[:2 * DIM].reshape(12, 128)),
        "vb": np.ascontiguousarray(bqkv[2 * DIM:].reshape(1, DIM)),
        "wp": np.ascontiguousarray(proj_w * WSCALE).astype(f8),
        "pb": proj_b.reshape(1, DIM).copy(),
        "w1": np.ascontiguousarray(W1 * WSCALE).astype(f8),
        "b1": np.ascontiguousarray(b1.reshape(24, 128)),
        "w2": mlp_w2.astype(bf),
        "b2": mlp_b2.reshape(1, DIM).copy(),
    }
    Eq = Eq.astype(f8).reshape(B, NW, NH, 28, N)
    Ek = Ek.astype(f8).reshape(B, NW, NH, 28, N)
    in_maps = []
    for b in range(B):
        m = dict(shared)
        m["x"] = np.ascontiguousarray(x[b].reshape(NTOK, DIM))
        m["eq"] = np.ascontiguousarray(Eq[b])
        m["ek"] = np.ascontiguousarray(Ek[b])
        in_maps.append(m)
    return in_maps


def kernel(**inputs):
    in_maps = _host_prep(inputs)
    if 'nc' not in _CACHE:
        _CACHE['pb_zero'] = not np.any(np.asarray(in_maps[0]['pb'], np.float32))
        _CACHE['b2_zero'] = not np.any(np.asarray(in_maps[0]['b2'], np.float32))
    nc = _build()
    trace = os.environ.get('KERNEL_TRACE') == '1'
    if trace:
        _install_ntff_hook()
    res = run_bass_kernel_spmd(nc, in_maps, list(range(B)), trace=trace)
    if trace and res.exec_time_ns is not None:
        print(f"HW exec time: {res.exec_time_ns} ns")
        _CACHE['exec_time_ns'] = res.exec_time_ns
    _CACHE['last_results'] = res
    out = np.stack([np.asarray(res.results[b]["y"]).reshape(HH, WW, DIM)
                    for b in range(B)])
    return out.astype(np.float32)


# revision 17
# speedup vs baseline: 1.0655x; 1.0655x over previous
"""Trainium2 Bass kernel for nn_Block_72138270704025 (windowed sparse attention
block: LN1 -> window partition -> MHA with decomposed rel-pos bias gathered by
q_idx/k_idx -> window unpartition -> residual -> LN2 -> MLP(gelu) -> residual).

Sharding: data-parallel over batch B=8, one batch element per NeuronCore; all
weights replicated.  Host folds LN affine params into the adjacent matmul
weights, precomputes the rel-pos tables Sh/Sw, and turns the per-(window,head)
index gathers into two small (28 x 196) operands per attention batch so the
bias folds into the logits matmul as a PSUM-accumulated matmul.

v2 optimizations over the bf16 baseline:
- fp8e4m3 DoubleRow matmuls (2 k-tiles per pass) for qkv/v/proj/fc1 and the
  PV+denominator matmuls (weights scaled x32 on host to clear the e4m3
  subnormal band; descale folded into the existing PSUM-evacuation ops).
- Softmax denominator reciprocal via the custom-DVE reciprocal_approx_fast
  (~5x cheaper than the iterative-divide microcode), one per head-PAIR: even
  head lands in PSUM rows 0:64, odd head rows 64:128 of one bank, so a single
  reciprocal + multiply normalizes both heads.
- Head-software-pipelining: QK/bias matmuls of head pair j+1 are emitted
  before the PV matmuls of pair j so the PE never stalls on the ACT-engine
  exp; keeps the HAM clock-gate at K=8/8 (the bf16 baseline ran most
  attention matmuls at half clock).
- Transposes as regular identity matmuls (counts as PE activity for HAM and
  is ~2.5x cheaper than transpose-mode).
- LN sqrt batched per band/group (one ACT sqrt per 7 tiles) to cut act-table
  set switches; rsqrt finished with reciprocal_approx_fast on DVE.
- Phase C (residual+LN2+MLP) groups interleaved into the band loop as their
  token ranges complete, removing the phase boundary bubble.
- Window gather/scatter as composite 3D DMAs (2 per window instead of 14)
  for interior windows; attn intermediate stored bf16.
"""
import os
import sys

for _p in ('/opt/trn_rl_repo', '/root/.axon_site/_ro/trn_rl_repo'):
    if os.path.isdir(_p) and _p not in sys.path:
        sys.path.append(_p)

import numpy as np
import ml_dtypes

import concourse.bass as bass
import concourse.tile as tile
from concourse import mybir
from concourse.bass_utils import run_bass_kernel_spmd
from concourse.tile import ScopedClock
from concourse.masks import make_identity

# ---- problem constants (hardcoded per contest rules) ----
B = 8
HH = 64
WW = 64
DIM = 768
NH = 12
WS = 14
HD = 64
N = 196            # tokens per window
NWS = 5            # window grid side
NW = 25            # windows per image
EPS = 1e-5
NTOK = HH * WW     # 4096 tokens per core
CH = 98            # window token chunk: 7 rows of 14 (196 = 2x98)
WSCALE = 32.0      # host fp8 weight scale (descaled at PSUM evacuation)

F32 = mybir.dt.float32
BF16 = mybir.dt.bfloat16
F8 = mybir.dt.float8e4
DR = mybir.MatmulPerfMode.DoubleRow


def _patch_tile_drain():
    """Walrus CoreV3 codegen rejects a Drain carrying multiple sem waits
    ("Too many sync wait commands").  Emit explicit wait_ge instructions
    before the kernel-tail drain instead."""
    if getattr(tile.TileContext, '_drain_patched', False):
        return

    def _drain_and_barrier(self, tick_clock, wait_clock):
        nc = self.nc
        dummy = nc.sync.nop(nofuse=True)
        wait_clock.add_sem_waits(dummy.ins, ScopedClock({None: tick_clock.global_clock}))
        waits = list(dummy.ins.sync_info.on_wait or [])
        dummy.ins.sync_info.on_wait = []
        assert self.sems is not None
        by_id = {}
        for h in self.sems.allocated().values():
            by_id[getattr(h, 'id', None)] = h
            by_id[getattr(h, 'name', None)] = h
        for w in waits:
            h = by_id.get(w.id) or by_id.get(w.ant_name)
            assert h is not None, (w.id, w.ant_name)
            nc.sync.wait_ge(h, w.wait_value)
        nc.sync.drain()
        nc.all_engine_barrier()
        popped = nc._tile_sem_poison_stack.pop()
        assert popped is self._sem_poison
        nc.clear_and_free_semaphores(list(self.sems.allocated().values()))
        nc.all_engine_barrier()

    tile.TileContext._drain_and_barrier = _drain_and_barrier
    tile.TileContext._drain_patched = True


def _install_ntff_hook():
    """Recreate the missing antenv.axon_hooks module so trace=True can reach
    the axon NTFF profiler (used only when KERNEL_TRACE=1)."""
    try:
        import types
        import antenv
        if 'antenv.axon_hooks' in sys.modules:
            return True
        mod = types.ModuleType('antenv.axon_hooks')
        mod._hook = None
        mod.set_axon_ntff_profile_hook = lambda h: setattr(mod, '_hook', h)
        mod.get_axon_ntff_profile_hook = lambda: mod._hook
        sys.modules['antenv.axon_hooks'] = mod
        antenv.axon_hooks = mod
        from trn_agent_boot.trn_boot import _ntff_profile_via_ctypes
        mod._hook = _ntff_profile_via_ctypes('/opt/axon/libaxon_pjrt.so')
        return mod._hook is not None
    except Exception:
        return False


# window geometry helpers
def _win_rc(w):
    return w // NWS, w % NWS


def _valid(w):
    wr, wc = _win_rc(w)
    return (14 if wr < 4 else 8), (14 if wc < 4 else 8)


_CACHE = {}


def _dedup_ldweights(nc):
    """Tile lowers each matmul to Ldweights+Matmult.  Back-to-back matmuls
    that share a stationary operand reload identical weights; drop the
    redundant Ldweights (keeping its sem waits / updates on a zero-cost
    EventSemaphore)."""
    ndrop = 0
    for fn in nc.m.functions:
        for blk in fn.blocks:
            insts = blk.instructions
            out = []
            prev_key = None
            dirty = False
            for ins in insts:
                if ins.engine != mybir.EngineType.PE:
                    out.append(ins)
                    continue
                if ins.opcode == 'Ldweights':
                    a = ins.ins[0]
                    key = (str(getattr(a, 'memory_location', None)),
                           getattr(a, 'offset', None), str(getattr(a, 'ap', None)),
                           str(getattr(ins, 'is_transpose', None)),
                           str(getattr(ins, 'perf_mode', None)))
                    si = ins.sync_info
                    has_sync = si and (si.on_wait or si.on_update)
                    if key == prev_key:
                        ndrop += 1
                        dirty = True
                        if has_sync:
                            ev = mybir.InstEventSemaphore(
                                name=f"LDDROP-{nc.next_id()}", ins=[], outs=[])
                            ev.engine = ins.engine
                            ev.sync_info = mybir.SyncInfo(
                                on_wait=list(si.on_wait or []),
                                on_update=list(si.on_update or []))
                            out.append(ev)
                        continue
                    prev_key = key
                    out.append(ins)
                elif ins.opcode == 'Matmult' and not getattr(ins, 'is_transpose', False):
                    out.append(ins)
                else:
                    prev_key = None
                    out.append(ins)
            if dirty:
                blk.instructions = out
    return ndrop


def _split_waits(nc, cap=None):
    """Walrus CoreV2/V3 codegen rejects instructions whose sync_info carries
    more waits than the per-opcode ISA ctrl struct holds.  Hoist excess waits
    onto standalone EventSemaphore instructions."""
    if cap is None:
        cap = int(os.environ.get('KERNEL_MAXWAITS', '1'))
    n_split = 0
    for fn in nc.m.functions:
        for blk in fn.blocks:
            insts = blk.instructions
            out = []
            dirty = False
            for ins in insts:
                si = ins.sync_info
                waits = list(si.on_wait) if si and si.on_wait else []
                limit = 1 if ins.opcode in ('Drain',) else cap
                if len(waits) > limit:
                    keep, extra = waits[:limit], waits[limit:]
                    for k in range(0, len(extra), cap):
                        ev = mybir.InstEventSemaphore(
                            name=f"WSPLIT-{nc.next_id()}", ins=[], outs=[])
                        ev.engine = ins.engine
                        ev.sync_info = mybir.SyncInfo(
                            on_wait=extra[k:k + cap], on_update=[])
                        out.append(ev)
                        n_split += 1
                    si.on_wait = keep
                    dirty = True
                out.append(ins)
            if dirty:
                blk.instructions = out
    return n_split


def _build():
    if 'nc' in _CACHE:
        return _CACHE['nc']
    _patch_tile_drain()

    nc = bass.Bass()

    # ---- dram parameters ----
    x_d = nc.dram_tensor("x", [NTOK, DIM], F32, kind="ExternalInput")
    eq_d = nc.dram_tensor("eq", [NW, NH, 28, N], F8, kind="ExternalInput")
    ek_d = nc.dram_tensor("ek", [NW, NH, 28, N], F8, kind="ExternalInput")
    wqk_d = nc.dram_tensor("wqk", [DIM, 2 * DIM], F8, kind="ExternalInput")
    wv_d = nc.dram_tensor("wv", [DIM, DIM], F8, kind="ExternalInput")
    bqk_d = nc.dram_tensor("bqk", [12, 128], F32, kind="ExternalInput")
    vb_d = nc.dram_tensor("vb", [1, DIM], F32, kind="ExternalInput")
    wp_d = nc.dram_tensor("wp", [DIM, DIM], F8, kind="ExternalInput")
    pb_d = nc.dram_tensor("pb", [1, DIM], F32, kind="ExternalInput")
    w1_d = nc.dram_tensor("w1", [DIM, 4 * DIM], F8, kind="ExternalInput")
    b1_d = nc.dram_tensor("b1", [24, 128], F32, kind="ExternalInput")
    w2_d = nc.dram_tensor("w2", [4 * DIM, DIM], BF16, kind="ExternalInput")
    b2_d = nc.dram_tensor("b2", [1, DIM], F32, kind="ExternalInput")
    y_d = nc.dram_tensor("y", [NTOK, DIM], F32, kind="ExternalOutput")

    dbg = os.environ.get('KERNEL_DEBUG') == '1'
    skind = dict(kind="ExternalOutput") if dbg else {}
    # xn1 banded by window row (7/7/7/7/4 token tiles)
    band_tiles = [7, 7, 7, 7, 4]
    xn1_b = [nc.dram_tensor(f"xn1b{i}", [band_tiles[i] * 128, DIM], F8)
             for i in range(5)]
    at_d = nc.dram_tensor("attn", [NTOK, DIM], BF16, **skind)

    x_t32 = x_d.rearrange("(a p) d -> a p d", p=128)      # 32 token tiles
    x_pt = x_d.rearrange("(a p) d -> p a d", p=128)       # grouped loads
    xn1b_t = [t.rearrange("(a p) d -> a p d", p=128) for t in xn1_b]
    xn1b_img = [t.rearrange("(r c) d -> r c d", c=WW) for t in xn1_b]
    at_img = at_d.rearrange("(r c) d -> r c d", c=WW)
    at_pt = at_d.rearrange("(a p) d -> p a d", p=128)
    y_t32 = y_d.rearrange("(a p) d -> a p d", p=128)

    inv_w = 1.0 / WSCALE

    with tile.TileContext(nc, pool_alloc_mode='queue') as tc:
        with tc.tile_pool(name="cW", bufs=1) as pcw, \
             tc.tile_pool(name="lnA", bufs=2) as pa, \
             tc.tile_pool(name="xtP", bufs=8) as pxt, \
             tc.tile_pool(name="xwP", bufs=3) as pxw, \
             tc.tile_pool(name="xwtP", bufs=3) as pxwt, \
             tc.tile_pool(name="qkP", bufs=2) as pqk, \
             tc.tile_pool(name="eqP", bufs=1) as peq, \
             tc.tile_pool(name="vP", bufs=2) as pv, \
             tc.tile_pool(name="hdP", bufs=4) as phd, \
             tc.tile_pool(name="owP", bufs=2) as pow_, \
             tc.tile_pool(name="gC", bufs=2) as pg, \
             tc.tile_pool(name="agC", bufs=1) as pag, \
             tc.tile_pool(name="yC", bufs=2) as py, \
             tc.tile_pool(name="hC", bufs=1) as ph, \
             tc.tile_pool(name="gX", bufs=2) as pgx, \
             tc.tile_pool(name="psB", bufs=6, space="PSUM") as psb, \
             tc.tile_pool(name="ptB", bufs=2, space="PSUM") as ptb:

            # ---- persistent weights / consts ----
            w1_sb = pcw.tile([128, 6, 4 * DIM], F8)
            nc.sync.dma_start(out=w1_sb[:], in_=w1_d.rearrange("(k p) n -> p k n", p=128))
            b1_sb = pcw.tile([128, 24], F32)
            nc.sync.dma_start(out=b1_sb[:], in_=b1_d.rearrange("a p -> p a"))
            if not _CACHE.get('b2_zero'):
                b2_sb = pcw.tile([128, DIM], F32)
                nc.gpsimd.dma_start(out=b2_sb[:], in_=b2_d[0:1, :].to_broadcast((128, DIM)))
            w2_sb = pcw.tile([128, 24, DIM], BF16)
            nc.sync.dma_start(out=w2_sb[:], in_=w2_d.rearrange("(k p) n -> p k n", p=128))
            eps_t = pcw.tile([128, 1], F32)
            nc.vector.memset(eps_t[:], EPS)
            ident = pcw.tile([128, 128], F8)
            make_identity(nc, ident[:])
            wqk_sb = pcw.tile([128, 6, 2 * DIM], F8)
            nc.sync.dma_start(out=wqk_sb[:], in_=wqk_d.rearrange("(k p) n -> p k n", p=128))
            wv_sb = pcw.tile([128, 6, DIM], F8)
            nc.sync.dma_start(out=wv_sb[:], in_=wv_d.rearrange("(k p) n -> p k n", p=128))
            wp_sb = pcw.tile([128, 6, DIM], F8)
            nc.sync.dma_start(out=wp_sb[:], in_=wp_d.rearrange("(k p) n -> p k n", p=128))
            bqk_sb = pcw.tile([128, 12], F32)
            nc.sync.dma_start(out=bqk_sb[:], in_=bqk_d.rearrange("a p -> p a"))
            if not _CACHE.get('vb_zero'):
                vb_sb = pcw.tile([128, DIM], F32)
                nc.gpsimd.dma_start(out=vb_sb[:], in_=vb_d[0:1, :].to_broadcast((128, DIM)))
            if not _CACHE.get('pb_zero'):
                pb_sb = pcw.tile([128, DIM], F32)
                nc.gpsimd.dma_start(out=pb_sb[:], in_=pb_d[0:1, :].to_broadcast((128, DIM)))

            sig_gelu = os.environ.get('KERNEL_GELU') == 'sig'

            def emit_ln1_band(band):
                """LN1 for this band's token tiles; batched sqrt for the band."""
                nbt = band_tiles[band]
                xts = []
                mvb = pa.tile([128, 2, 7], F32, tag="mvb")
                for bt in range(nbt):
                    t = band * 7 + bt
                    xt = pxt.tile([128, DIM], F32, tag="xt")
                    nc.sync.dma_start(out=xt[:], in_=x_t32[t])
                    st = pa.tile([128, 2, 6], F32, tag="st")
                    for s in range(2):
                        nc.vector.bn_stats(out=st[:, s, :], in_=xt[:, s * 384:(s + 1) * 384])
                    nc.vector.bn_aggr(out=mvb[:, :, bt], in_=st[:])
                    xts.append(xt)
                sdb = pa.tile([128, 7], F32, tag="sdb")
                nc.scalar.activation(out=sdb[:, 0:nbt], in_=mvb[:, 1, 0:nbt],
                                     func=mybir.ActivationFunctionType.Sqrt,
                                     bias=eps_t[:], scale=1.0)
                rsd = pa.tile([128, 7], F32, tag="rsd")
                nc.vector.reciprocal(out=rsd[:, 0:nbt], in_=sdb[:, 0:nbt])
                for bt in range(nbt):
                    xn = pa.tile([128, DIM], F8, tag="xn")
                    nc.vector.tensor_scalar(out=xn[:], in0=xts[bt][:],
                                            scalar1=mvb[:, 0, bt:bt + 1],
                                            scalar2=rsd[:, bt:bt + 1],
                                            op0=mybir.AluOpType.subtract,
                                            op1=mybir.AluOpType.mult)
                    nc.sync.dma_start(out=xn1b_t[band][bt], in_=xn[:])

            def emit_window_group(wins):
                """One group (pair or lone window): qkv, per-window V + pipelined
                heads + proj + scatter."""
                nwin = len(wins)
                wfree = N * nwin
                FPAD = 400 if nwin == 2 else 208   # fp8 Ko-step must be %16
                xwtb = pxwt.tile([128, 6, FPAD], F8, tag="xwtb")
                qkt = pqk.tile([128, 12, wfree], F8, tag="qkt")
                att = pxwt.tile([128, 6, FPAD], F8, tag="att")

                # gather + transpose into xwtb
                for ww_i, w in enumerate(wins):
                    woff = ww_i * N
                    wr, wc = _win_rc(w)
                    vr, vc = _valid(w)
                    edge = (vr < 14) or (vc < 14)
                    xw = pxw.tile([128, 2, DIM], F8, tag="xw")
                    if edge:
                        nc.gpsimd.memset(xw[0:CH, 0, :], 0.0)
                        nc.gpsimd.memset(xw[0:CH, 1, :], 0.0)
                        for r in range(vr):
                            c, p0 = r // 7, (r % 7) * WS
                            nc.gpsimd.dma_start(
                                out=xw[p0:p0 + vc, c, :],
                                in_=xn1b_img[wr][r, wc * WS:wc * WS + vc, :])
                    else:
                        for c in range(2):
                            nc.gpsimd.dma_start(
                                out=xw[0:CH, c, :],
                                in_=xn1b_img[wr][c * 7:c * 7 + 7,
                                                 wc * WS:wc * WS + WS, :])
                    # transpose via regular identity matmul: out = xw_slice.T @ I
                    for c, coff in ((0, 0), (1, CH)):
                        for j in range(6):
                            pt = ptb.tile([128, 128], F32, tag="pt")
                            nc.tensor.matmul(
                                pt[0:128, 0:CH],
                                lhsT=xw[0:CH, c, j * 128:(j + 1) * 128],
                                rhs=ident[0:CH, 0:CH],
                                start=True, stop=True)
                            nc.vector.tensor_copy(
                                out=xwtb[:, j, woff + coff:woff + coff + CH],
                                in_=pt[0:128, 0:CH])

                # qkv^T for the whole group (fp8 DoubleRow over k-tile pairs)
                for oc in range(12):
                    pqm = psb.tile([128, 392], F32, tag="ps")
                    for kp in range(3):
                        nc.tensor.matmul(
                            pqm[:, 0:wfree],
                            lhsT=wqk_sb[:, 2 * kp:2 * kp + 2, oc * 128:(oc + 1) * 128],
                            rhs=xwtb[:, 2 * kp:2 * kp + 2, 0:wfree],
                            perf_mode=DR,
                            start=(kp == 0), stop=(kp == 2))
                    nc.vector.tensor_scalar(out=qkt[:, oc, :], in0=pqm[:, 0:wfree],
                                            scalar1=inv_w,
                                            scalar2=bqk_sb[:, oc:oc + 1],
                                            op0=mybir.AluOpType.mult,
                                            op1=mybir.AluOpType.add)

                for ww_i, w in enumerate(wins):
                    woff = ww_i * N
                    # V (fp8): all heads + 64 ones columns for the denominator
                    va = pv.tile([128, 2, DIM + 64], F8, tag="va")
                    for c, coff in ((0, 0), (1, CH)):
                        nc.gpsimd.memset(va[0:CH, c, DIM:DIM + 64], 1.0)
                        pv0 = psb.tile([128, 384], F32, tag="ps")
                        pv1 = psb.tile([128, 384], F32, tag="ps")
                        for kp in range(3):
                            nc.tensor.matmul(
                                pv0[0:CH, :],
                                lhsT=xwtb[:, 2 * kp:2 * kp + 2,
                                          woff + coff:woff + coff + CH],
                                rhs=wv_sb[:, 2 * kp:2 * kp + 2, 0:384],
                                perf_mode=DR,
                                start=(kp == 0), stop=(kp == 2))
                            nc.tensor.matmul(
                                pv1[0:CH, :],
                                lhsT=xwtb[:, 2 * kp:2 * kp + 2,
                                          woff + coff:woff + coff + CH],
                                rhs=wv_sb[:, 2 * kp:2 * kp + 2, 384:768],
                                perf_mode=DR,
                                start=(kp == 0), stop=(kp == 2))
                        for half, pvm in ((0, pv0), (1, pv1)):
                            if _CACHE.get('vb_zero'):
                                nc.vector.tensor_scalar(
                                    out=va[0:CH, c, half * 384:(half + 1) * 384],
                                    in0=pvm[0:CH, :], scalar1=inv_w, scalar2=None,
                                    op0=mybir.AluOpType.mult)
                            else:
                                nc.vector.scalar_tensor_tensor(
                                    out=va[0:CH, c, half * 384:(half + 1) * 384],
                                    in0=pvm[0:CH, :],
                                    scalar=inv_w,
                                    in1=vb_sb[0:CH, half * 384:(half + 1) * 384],
                                    op0=mybir.AluOpType.mult,
                                    op1=mybir.AluOpType.add)

                    eqt = peq.tile([28, NH, N], F8, tag="eqt")
                    nc.sync.dma_start(out=eqt[:], in_=eq_d[w].rearrange("h r i -> r h i"))
                    ekt = peq.tile([28, NH, N], F8, tag="ekt")
                    nc.sync.dma_start(out=ekt[:], in_=ek_d[w].rearrange("h r i -> r h i"))

                    # heads: QK/bias+exp for pair p, then PV/normalize for pair p-1
                    pTs = {}
                    psos = {}

                    def emit_qk(h):
                        hp = (h % 2) * 64
                        qT = qkt[hp:hp + 64, h // 2, woff:woff + N]
                        kT = qkt[hp:hp + 64, 6 + h // 2, woff:woff + N]
                        pss = psb.tile([128, 2 * N], F32, tag="ps")
                        for c in range(2):
                            nc.tensor.matmul(pss[0:CH, c * N:(c + 1) * N],
                                             lhsT=kT[:, c * CH:(c + 1) * CH], rhs=qT,
                                             start=True, stop=False)
                            nc.tensor.matmul(pss[0:CH, c * N:(c + 1) * N],
                                             lhsT=ekt[:, h, c * CH:(c + 1) * CH],
                                             rhs=eqt[:, h, :],
                                             start=False, stop=True)
                        pT = phd.tile([128, 2, 208], F8, tag="pT")
                        nc.scalar.activation(out=pT[0:CH, :, 0:N], in_=pss[0:CH, 0:2 * N],
                                             func=mybir.ActivationFunctionType.Exp)
                        pTs[h] = pT

                    def emit_pv(p):
                        pso = psb.tile([128, 2 * N], F32, tag="ps")
                        for h in (2 * p, 2 * p + 1):
                            b0 = (h % 2) * 64
                            pT = pTs.pop(h)
                            if b0 == 0:
                                # DoubleRow folds both key-chunks into one pass
                                nc.tensor.matmul(pso[0:64, 0:N],
                                                 lhsT=va[0:CH, 0:2, h * 64:(h + 1) * 64],
                                                 rhs=pT[0:CH, 0:2, 0:N],
                                                 perf_mode=DR, start=True, stop=True,
                                                 skip_group_check=True)
                                nc.tensor.matmul(pso[0:64, N:2 * N],
                                                 lhsT=va[0:CH, 0:2, DIM:DIM + 64],
                                                 rhs=pT[0:CH, 0:2, 0:N],
                                                 perf_mode=DR, start=True, stop=True,
                                                 skip_group_check=True)
                            else:
                                # walrus rejects DoubleRow + col-offset
                                # tile_position; plain fp8 per chunk instead
                                for c in range(2):
                                    nc.tensor.matmul(pso[64:128, 0:N],
                                                     lhsT=va[0:CH, c, h * 64:(h + 1) * 64],
                                                     rhs=pT[0:CH, c, 0:N],
                                                     start=(c == 0), stop=(c == 1),
                                                     skip_group_check=True)
                                    nc.tensor.matmul(pso[64:128, N:2 * N],
                                                     lhsT=va[0:CH, c, DIM:DIM + 64],
                                                     rhs=pT[0:CH, c, 0:N],
                                                     start=(c == 0), stop=(c == 1),
                                                     skip_group_check=True)
                        psos[p] = pso

                    def emit_norm(p):
                        pso = psos.pop(p)
                        rb = phd.tile([128, N], F32, tag="rb")
                        nc.vector.reciprocal(out=rb[:], in_=pso[:, N:2 * N])
                        nc.vector.tensor_mul(out=att[:, p, woff:woff + N],
                                             in0=pso[:, 0:N], in1=rb[:])

                    for p in range(6):
                        emit_qk(2 * p)
                        emit_qk(2 * p + 1)
                        if p >= 1:
                            emit_pv(p - 1)
                            emit_norm(p - 1)
                    emit_pv(5)
                    emit_norm(5)

                    # proj (fp8 DoubleRow) -> ow, then unpartition scatter
                    ow = pow_.tile([128, 2, DIM], BF16, tag="ow")
                    for c, coff in ((0, 0), (1, CH)):
                        pp0 = psb.tile([128, 384], F32, tag="ps")
                        pp1 = psb.tile([128, 384], F32, tag="ps")
                        for kp in range(3):
                            nc.tensor.matmul(
                                pp0[0:CH, :],
                                lhsT=att[:, 2 * kp:2 * kp + 2,
                                         woff + coff:woff + coff + CH],
                                rhs=wp_sb[:, 2 * kp:2 * kp + 2, 0:384],
                                perf_mode=DR,
                                start=(kp == 0), stop=(kp == 2))
                            nc.tensor.matmul(
                                pp1[0:CH, :],
                                lhsT=att[:, 2 * kp:2 * kp + 2,
                                         woff + coff:woff + coff + CH],
                                rhs=wp_sb[:, 2 * kp:2 * kp + 2, 384:768],
                                perf_mode=DR,
                                start=(kp == 0), stop=(kp == 2))
                        for half, psp in ((0, pp0), (1, pp1)):
                            if _CACHE.get('pb_zero'):
                                nc.scalar.activation(
                                    out=ow[0:CH, c, half * 384:(half + 1) * 384],
                                    in_=psp[0:CH, :],
                                    func=mybir.ActivationFunctionType.Copy,
                                    bias=0.0, scale=inv_w)
                            else:
                                nc.vector.scalar_tensor_tensor(
                                    out=ow[0:CH, c, half * 384:(half + 1) * 384],
                                    in0=psp[0:CH, :], scalar=inv_w,
                                    in1=pb_sb[0:CH, half * 384:(half + 1) * 384],
                                    op0=mybir.AluOpType.mult,
                                    op1=mybir.AluOpType.add)
                    wr, wc = _win_rc(w)
                    vr, vc = _valid(w)
                    if vr == 14 and vc == 14:
                        for c in range(2):
                            nc.gpsimd.dma_start(
                                out=at_img[wr * WS + c * 7:wr * WS + c * 7 + 7,
                                           wc * WS:wc * WS + WS, :],
                                in_=ow[0:CH, c, :])
                    else:
                        for r in range(vr):
                            c, p0 = r // 7, (r % 7) * WS
                            nc.gpsimd.dma_start(
                                out=at_img[wr * WS + r, wc * WS:wc * WS + vc, :],
                                in_=ow[p0:p0 + vc, c, :])

            def emit_c_group(g):
                """Phase C for token tiles 2g..2g+1: residual, LN2, MLP, out."""
                xg = pg.tile([128, 2, DIM], F32, tag="xg")
                ag = pag.tile([128, 2, DIM], BF16, tag="ag")
                nc.sync.dma_start(out=xg[:], in_=x_pt[:, 2 * g:2 * g + 2, :])
                nc.sync.dma_start(out=ag[:], in_=at_pt[:, 2 * g:2 * g + 2, :])
                # x2 = x + attn (in place into xg)
                nc.vector.tensor_add(out=xg[:, :, :], in0=xg[:, :, :], in1=ag[:, :, :])
                xn2t = pgx.tile([128, 6, 256], F8, tag="xn2t")
                mvc = pg.tile([128, 2, 2], F32, tag="mvc")
                for s in range(2):
                    st = pg.tile([128, 2, 6], F32, tag="stC")
                    for sub in range(2):
                        nc.vector.bn_stats(out=st[:, sub, :],
                                           in_=xg[:, s, sub * 384:(sub + 1) * 384])
                    nc.vector.bn_aggr(out=mvc[:, :, s], in_=st[:])
                sdc = pg.tile([128, 2], F32, tag="sdC")
                nc.scalar.activation(out=sdc[:], in_=mvc[:, 1, :],
                                     func=mybir.ActivationFunctionType.Sqrt,
                                     bias=eps_t[:], scale=1.0)
                rsc = pg.tile([128, 2], F32, tag="rsC")
                nc.vector.reciprocal(out=rsc[:], in_=sdc[:])
                for s in range(2):
                    xn2b = pg.tile([128, DIM], F8, tag="xn2b")
                    nc.vector.tensor_scalar(out=xn2b[:, :], in0=xg[:, s, :],
                                            scalar1=mvc[:, 0, s:s + 1],
                                            scalar2=rsc[:, s:s + 1],
                                            op0=mybir.AluOpType.subtract,
                                            op1=mybir.AluOpType.mult)
                    if not _CACHE.get('b2_zero'):
                        nc.vector.tensor_add(out=xg[:, s, :], in0=xg[:, s, :],
                                             in1=b2_sb[:])
                    for j in range(6):
                        pt = ptb.tile([128, 128], F32, tag="pt")
                        nc.tensor.matmul(pt[:, :],
                                         lhsT=xn2b[:, j * 128:(j + 1) * 128],
                                         rhs=ident[:, :], start=True, stop=True)
                        nc.vector.tensor_copy(out=xn2t[:, j, s * 128:(s + 1) * 128],
                                              in_=pt[:, :])
                h1t = ph.tile([128, 24, 256], BF16, tag="h1t")
                for oc in range(24):
                    psh = psb.tile([128, 256], F32, tag="ps")
                    for kp in range(3):
                        nc.tensor.matmul(
                            psh[:, :],
                            lhsT=w1_sb[:, 2 * kp:2 * kp + 2, oc * 128:(oc + 1) * 128],
                            rhs=xn2t[:, 2 * kp:2 * kp + 2, :],
                            perf_mode=DR,
                            start=(kp == 0), stop=(kp == 2))
                    if sig_gelu:
                        # CoreSim lacks Gelu; x*sigmoid(1.702x) validates shapes
                        hpre = pg.tile([128, 256], F32, tag="hpre")
                        nc.scalar.activation(out=hpre[:], in_=psh[:, :],
                                             func=mybir.ActivationFunctionType.Identity,
                                             bias=b1_sb[:, oc:oc + 1], scale=inv_w)
                        sg = pg.tile([128, 256], F32, tag="sg")
                        nc.scalar.activation(out=sg[:], in_=hpre[:],
                                             func=mybir.ActivationFunctionType.Sigmoid,
                                             bias=0.0, scale=1.702)
                        nc.vector.tensor_mul(out=h1t[:, oc, :], in0=hpre[:], in1=sg[:])
                    else:
                        nc.scalar.activation(out=h1t[:, oc, :], in_=psh[:, :],
                                             func=mybir.ActivationFunctionType.Gelu,
                                             bias=b1_sb[:, oc:oc + 1], scale=inv_w)
                for s in range(2):
                    pf0 = psb.tile([128, 384], F32, tag="ps")
                    pf1 = psb.tile([128, 384], F32, tag="ps")
                    for kt in range(24):
                        nc.tensor.matmul(
                            pf0[:, :],
                            lhsT=h1t[:, kt, s * 128:(s + 1) * 128],
                            rhs=w2_sb[:, kt, 0:384],
                            start=(kt == 0), stop=(kt == 23))
                        nc.tensor.matmul(
                            pf1[:, :],
                            lhsT=h1t[:, kt, s * 128:(s + 1) * 128],
                            rhs=w2_sb[:, kt, 384:768],
                            start=(kt == 0), stop=(kt == 23))
                    yo = py.tile([128, DIM], F32, tag="yo")
                    for half, psf in ((0, pf0), (1, pf1)):
                        nc.vector.tensor_add(
                            out=yo[:, half * 384:(half + 1) * 384],
                            in0=psf[:, :],
                            in1=xg[:, s, half * 384:(half + 1) * 384])
                    nc.sync.dma_start(out=y_t32[2 * g + s], in_=yo[:])

            # groups of phase C that become ready after each band completes:
            # band b covers image rows 14b..14b+13 -> tokens to (14b+14)*64
            c_ready = {0: [0, 1, 2], 1: [3, 4, 5, 6], 2: [7, 8, 9],
                       3: [10, 11, 12, 13], 4: [14, 15]}

            emit_ln1_band(0)
            for band in range(5):
                w0 = band * NWS
                emit_window_group((w0, w0 + 1))
                # overlap next band's LN1 (DVE/DMA) with this band's windows
                if band < 4:
                    emit_ln1_band(band + 1)
                emit_window_group((w0 + 2, w0 + 3))
                emit_window_group((w0 + 4,))
                for g in c_ready[band]:
                    emit_c_group(g)

    if os.environ.get('KERNEL_NOLDDEDUP') != '1':
        _dedup_ldweights(nc)
    if os.environ.get('KERNEL_SIM') != '1':
        _split_waits(nc)
    _CACHE['nc'] = nc
    return nc


def _host_prep(inputs):
    """Fold LN affines into matmul weights, build rel-pos operands."""
    f32 = np.float32
    x = np.asarray(inputs['x'], f32)
    q_idx = np.asarray(inputs['q_idx']).astype(np.int64)
    k_idx = np.asarray(inputs['k_idx']).astype(np.int64)
    ln1_w = np.asarray(inputs['ln1_w'], f32); ln1_b = np.asarray(inputs['ln1_b'], f32)
    ln2_w = np.asarray(inputs['ln2_w'], f32); ln2_b = np.asarray(inputs['ln2_b'], f32)
    qkv_w = np.asarray(inputs['qkv_w'], f32); qkv_b = np.asarray(inputs['qkv_b'], f32)
    proj_w = np.asarray(inputs['proj_w'], f32); proj_b = np.asarray(inputs['proj_b'], f32)
    mlp_w1 = np.asarray(inputs['mlp_w1'], f32); mlp_b1 = np.asarray(inputs['mlp_b1'], f32)
    mlp_w2 = np.asarray(inputs['mlp_w2'], f32); mlp_b2 = np.asarray(inputs['mlp_b2'], f32)
    rel_h = np.asarray(inputs['rel_h'], f32); rel_w = np.asarray(inputs['rel_w'], f32)

    scale = HD ** -0.5
    Wqkv = ln1_w[:, None] * qkv_w
    bqkv = ln1_b @ qkv_w + qkv_b
    Wqkv = Wqkv.copy(); bqkv = bqkv.copy()
    Wqkv[:, :DIM] *= scale
    bqkv[:DIM] *= scale
    W1 = ln2_w[:, None] * mlp_w1
    b1 = ln2_b @ mlp_w1 + mlp_b1

    coords = np.arange(WS)[:, None] - np.arange(WS)[None, :] + (WS - 1)
    Sh = rel_h[coords].sum(-1).astype(f32)
    Sw = rel_w[coords].sum(-1).astype(f32)

    qr, qc = q_idx // WS, q_idx % WS
    kr, kc = k_idx // WS, k_idx % WS
    nb = q_idx.shape[0]
    Eq = np.concatenate([np.take(Sh, qr, axis=0).transpose(0, 2, 1),
                         np.take(Sw, qc, axis=0).transpose(0, 2, 1)], axis=1)
    Ek = np.zeros((nb, 28, N), f32)
    bi = np.arange(nb)[:, None]
    ar = np.arange(N)[None, :]
    Ek[bi, kr, ar] = 1.0
    Ek[bi, WS + kc, ar] = 1.0

    bf = ml_dtypes.bfloat16
    f8 = ml_dtypes.float8_e4m3fn
    shared = {
        "wqk": np.ascontiguousarray(Wqkv[:, :2 * DIM] * WSCALE).astype(f8),
        "wv": np.ascontiguousarray(Wqkv[:, 2 * DIM:] * WSCALE).astype(f8),
        "bqk": np.ascontiguousarray(bqkv[:2 * DIM].reshape(12, 128)),
        "vb": np.ascontiguousarray(bqkv[2 * DIM:].reshape(1, DIM)),
        "wp": np.ascontiguousarray(proj_w * WSCALE).astype(f8),
        "pb": proj_b.reshape(1, DIM).copy(),
        "w1": np.ascontiguousarray(W1 * WSCALE).astype(f8),
        "b1": np.ascontiguousarray(b1.reshape(24, 128)),
        "w2": mlp_w2.astype(bf),
        "b2": mlp_b2.reshape(1, DIM).copy(),
    }
    Eq = Eq.astype(f8).reshape(B, NW, NH, 28, N)
    Ek = Ek.astype(f8).reshape(B, NW, NH, 28, N)
    in_maps = []
    for b in range(B):
        m = dict(shared)
        m["x"] = np.ascontiguousarray(x[b].reshape(NTOK, DIM))
        m["eq"] = np.ascontiguousarray(Eq[b])
        m["ek"] = np.ascontiguousarray(Ek[b])
        in_maps.append(m)
    return in_maps


def kernel(**inputs):
    in_maps = _host_prep(inputs)
    if 'nc' not in _CACHE:
        _CACHE['pb_zero'] = not np.any(np.asarray(in_maps[0]['pb'], np.float32))
        _CACHE['b2_zero'] = not np.any(np.asarray(in_maps[0]['b2'], np.float32))
    nc = _build()
    trace = os.environ.get('KERNEL_TRACE') == '1'
    if trace:
        _install_ntff_hook()
    res = run_bass_kernel_spmd(nc, in_maps, list(range(B)), trace=trace)
    if trace and res.exec_time_ns is not None:
        print(f"HW exec time: {res.exec_time_ns} ns")
        _CACHE['exec_time_ns'] = res.exec_time_ns
    _CACHE['last_results'] = res
    out = np.stack([np.asarray(res.results[b]["y"]).reshape(HH, WW, DIM)
                    for b in range(B)])
    return out.astype(np.float32)


# revision 18
# speedup vs baseline: 1.0739x; 1.0079x over previous
"""Trainium2 Bass kernel for nn_Block_72138270704025 (windowed sparse attention
block: LN1 -> window partition -> MHA with decomposed rel-pos bias gathered by
q_idx/k_idx -> window unpartition -> residual -> LN2 -> MLP(gelu) -> residual).

Sharding: data-parallel over batch B=8, one batch element per NeuronCore; all
weights replicated.  Host folds LN affine params into the adjacent matmul
weights, precomputes the rel-pos tables Sh/Sw, and turns the per-(window,head)
index gathers into two small (28 x 196) operands per attention batch so the
bias folds into the logits matmul as a PSUM-accumulated matmul.

v2 optimizations over the bf16 baseline:
- fp8e4m3 DoubleRow matmuls (2 k-tiles per pass) for qkv/v/proj/fc1 and the
  PV+denominator matmuls (weights scaled x32 on host to clear the e4m3
  subnormal band; descale folded into the existing PSUM-evacuation ops).
- Softmax denominator reciprocal via the custom-DVE reciprocal_approx_fast
  (~5x cheaper than the iterative-divide microcode), one per head-PAIR: even
  head lands in PSUM rows 0:64, odd head rows 64:128 of one bank, so a single
  reciprocal + multiply normalizes both heads.
- Head-software-pipelining: QK/bias matmuls of head pair j+1 are emitted
  before the PV matmuls of pair j so the PE never stalls on the ACT-engine
  exp; keeps the HAM clock-gate at K=8/8 (the bf16 baseline ran most
  attention matmuls at half clock).
- Transposes as regular identity matmuls (counts as PE activity for HAM and
  is ~2.5x cheaper than transpose-mode).
- LN sqrt batched per band/group (one ACT sqrt per 7 tiles) to cut act-table
  set switches; rsqrt finished with reciprocal_approx_fast on DVE.
- Phase C (residual+LN2+MLP) groups interleaved into the band loop as their
  token ranges complete, removing the phase boundary bubble.
- Window gather/scatter as composite 3D DMAs (2 per window instead of 14)
  for interior windows; attn intermediate stored bf16.
"""
import os
import sys

for _p in ('/opt/trn_rl_repo', '/root/.axon_site/_ro/trn_rl_repo'):
    if os.path.isdir(_p) and _p not in sys.path:
        sys.path.append(_p)

import numpy as np
import ml_dtypes

import concourse.bass as bass
import concourse.tile as tile
from concourse import mybir
from concourse.bass_utils import run_bass_kernel_spmd
from concourse.tile import ScopedClock
from concourse.masks import make_identity

# ---- problem constants (hardcoded per contest rules) ----
B = 8
HH = 64
WW = 64
DIM = 768
NH = 12
WS = 14
HD = 64
N = 196            # tokens per window
NWS = 5            # window grid side
NW = 25            # windows per image
EPS = 1e-5
NTOK = HH * WW     # 4096 tokens per core
CH = 98            # window token chunk: 7 rows of 14 (196 = 2x98)
WSCALE = 32.0      # host fp8 weight scale (descaled at PSUM evacuation)

F32 = mybir.dt.float32
BF16 = mybir.dt.bfloat16
F8 = mybir.dt.float8e4
DR = mybir.MatmulPerfMode.DoubleRow


def _patch_tile_drain():
    """Walrus CoreV3 codegen rejects a Drain carrying multiple sem waits
    ("Too many sync wait commands").  Emit explicit wait_ge instructions
    before the kernel-tail drain instead."""
    if getattr(tile.TileContext, '_drain_patched', False):
        return

    def _drain_and_barrier(self, tick_clock, wait_clock):
        nc = self.nc
        dummy = nc.sync.nop(nofuse=True)
        wait_clock.add_sem_waits(dummy.ins, ScopedClock({None: tick_clock.global_clock}))
        waits = list(dummy.ins.sync_info.on_wait or [])
        dummy.ins.sync_info.on_wait = []
        assert self.sems is not None
        by_id = {}
        for h in self.sems.allocated().values():
            by_id[getattr(h, 'id', None)] = h
            by_id[getattr(h, 'name', None)] = h
        for w in waits:
            h = by_id.get(w.id) or by_id.get(w.ant_name)
            assert h is not None, (w.id, w.ant_name)
            nc.sync.wait_ge(h, w.wait_value)
        nc.sync.drain()
        nc.all_engine_barrier()
        popped = nc._tile_sem_poison_stack.pop()
        assert popped is self._sem_poison
        nc.clear_and_free_semaphores(list(self.sems.allocated().values()))
        nc.all_engine_barrier()

    tile.TileContext._drain_and_barrier = _drain_and_barrier
    tile.TileContext._drain_patched = True


def _install_ntff_hook():
    """Recreate the missing antenv.axon_hooks module so trace=True can reach
    the axon NTFF profiler (used only when KERNEL_TRACE=1)."""
    try:
        import types
        import antenv
        if 'antenv.axon_hooks' in sys.modules:
            return True
        mod = types.ModuleType('antenv.axon_hooks')
        mod._hook = None
        mod.set_axon_ntff_profile_hook = lambda h: setattr(mod, '_hook', h)
        mod.get_axon_ntff_profile_hook = lambda: mod._hook
        sys.modules['antenv.axon_hooks'] = mod
        antenv.axon_hooks = mod
        from trn_agent_boot.trn_boot import _ntff_profile_via_ctypes
        mod._hook = _ntff_profile_via_ctypes('/opt/axon/libaxon_pjrt.so')
        return mod._hook is not None
    except Exception:
        return False


# window geometry helpers
def _win_rc(w):
    return w // NWS, w % NWS


def _valid(w):
    wr, wc = _win_rc(w)
    return (14 if wr < 4 else 8), (14 if wc < 4 else 8)


_CACHE = {}


def _dedup_ldweights(nc):
    """Tile lowers each matmul to Ldweights+Matmult.  Back-to-back matmuls
    that share a stationary operand reload identical weights; drop the
    redundant Ldweights (keeping its sem waits / updates on a zero-cost
    EventSemaphore)."""
    ndrop = 0
    for fn in nc.m.functions:
        for blk in fn.blocks:
            insts = blk.instructions
            out = []
            prev_key = None
            dirty = False
            for ins in insts:
                if ins.engine != mybir.EngineType.PE:
                    out.append(ins)
                    continue
                if ins.opcode == 'Ldweights':
                    a = ins.ins[0]
                    key = (str(getattr(a, 'memory_location', None)),
                           getattr(a, 'offset', None), str(getattr(a, 'ap', None)),
                           str(getattr(ins, 'is_transpose', None)),
                           str(getattr(ins, 'perf_mode', None)))
                    si = ins.sync_info
                    has_sync = si and (si.on_wait or si.on_update)
                    if key == prev_key:
                        ndrop += 1
                        dirty = True
                        if has_sync:
                            ev = mybir.InstEventSemaphore(
                                name=f"LDDROP-{nc.next_id()}", ins=[], outs=[])
                            ev.engine = ins.engine
                            ev.sync_info = mybir.SyncInfo(
                                on_wait=list(si.on_wait or []),
                                on_update=list(si.on_update or []))
                            out.append(ev)
                        continue
                    prev_key = key
                    out.append(ins)
                elif ins.opcode == 'Matmult' and not getattr(ins, 'is_transpose', False):
                    out.append(ins)
                else:
                    prev_key = None
                    out.append(ins)
            if dirty:
                blk.instructions = out
    return ndrop


def _split_waits(nc, cap=None):
    """Walrus CoreV2/V3 codegen rejects instructions whose sync_info carries
    more waits than the per-opcode ISA ctrl struct holds.  Hoist excess waits
    onto standalone EventSemaphore instructions."""
    if cap is None:
        cap = int(os.environ.get('KERNEL_MAXWAITS', '1'))
    n_split = 0
    for fn in nc.m.functions:
        for blk in fn.blocks:
            insts = blk.instructions
            out = []
            dirty = False
            for ins in insts:
                si = ins.sync_info
                waits = list(si.on_wait) if si and si.on_wait else []
                limit = 1 if ins.opcode in ('Drain',) else cap
                if len(waits) > limit:
                    keep, extra = waits[:limit], waits[limit:]
                    for k in range(0, len(extra), cap):
                        ev = mybir.InstEventSemaphore(
                            name=f"WSPLIT-{nc.next_id()}", ins=[], outs=[])
                        ev.engine = ins.engine
                        ev.sync_info = mybir.SyncInfo(
                            on_wait=extra[k:k + cap], on_update=[])
                        out.append(ev)
                        n_split += 1
                    si.on_wait = keep
                    dirty = True
                out.append(ins)
            if dirty:
                blk.instructions = out
    return n_split


def _build():
    if 'nc' in _CACHE:
        return _CACHE['nc']
    _patch_tile_drain()

    nc = bass.Bass()

    # ---- dram parameters ----
    x_d = nc.dram_tensor("x", [NTOK, DIM], F32, kind="ExternalInput")
    eq_d = nc.dram_tensor("eq", [NW, NH, 28, N], F8, kind="ExternalInput")
    ek_d = nc.dram_tensor("ek", [NW, NH, 28, N], F8, kind="ExternalInput")
    wqk_d = nc.dram_tensor("wqk", [DIM, 2 * DIM], F8, kind="ExternalInput")
    wv_d = nc.dram_tensor("wv", [DIM, DIM], F8, kind="ExternalInput")
    bqk_d = nc.dram_tensor("bqk", [12, 128], F32, kind="ExternalInput")
    vb_d = nc.dram_tensor("vb", [1, DIM], F32, kind="ExternalInput")
    wp_d = nc.dram_tensor("wp", [DIM, DIM], F8, kind="ExternalInput")
    pb_d = nc.dram_tensor("pb", [1, DIM], F32, kind="ExternalInput")
    w1_d = nc.dram_tensor("w1", [DIM, 4 * DIM], F8, kind="ExternalInput")
    b1_d = nc.dram_tensor("b1", [24, 128], F32, kind="ExternalInput")
    w2_d = nc.dram_tensor("w2", [4 * DIM, DIM], BF16, kind="ExternalInput")
    b2_d = nc.dram_tensor("b2", [1, DIM], F32, kind="ExternalInput")
    y_d = nc.dram_tensor("y", [NTOK, DIM], F32, kind="ExternalOutput")

    dbg = os.environ.get('KERNEL_DEBUG') == '1'
    skind = dict(kind="ExternalOutput") if dbg else {}
    # xn1 banded by window row (7/7/7/7/4 token tiles)
    band_tiles = [7, 7, 7, 7, 4]
    xn1_b = [nc.dram_tensor(f"xn1b{i}", [band_tiles[i] * 128, DIM], F8)
             for i in range(5)]
    at_d = nc.dram_tensor("attn", [NTOK, DIM], BF16, **skind)

    x_t32 = x_d.rearrange("(a p) d -> a p d", p=128)      # 32 token tiles
    x_pt = x_d.rearrange("(a p) d -> p a d", p=128)       # grouped loads
    xn1b_t = [t.rearrange("(a p) d -> a p d", p=128) for t in xn1_b]
    xn1b_img = [t.rearrange("(r c) d -> r c d", c=WW) for t in xn1_b]
    at_img = at_d.rearrange("(r c) d -> r c d", c=WW)
    at_pt = at_d.rearrange("(a p) d -> p a d", p=128)
    y_t32 = y_d.rearrange("(a p) d -> a p d", p=128)

    inv_w = 1.0 / WSCALE

    with tile.TileContext(nc, pool_alloc_mode='queue') as tc:
        with tc.tile_pool(name="cW", bufs=1) as pcw, \
             tc.tile_pool(name="lnA", bufs=2) as pa, \
             tc.tile_pool(name="xtP", bufs=8) as pxt, \
             tc.tile_pool(name="xwP", bufs=2) as pxw, \
             tc.tile_pool(name="xwtP", bufs=2) as pxwt, \
             tc.tile_pool(name="qkP", bufs=2) as pqk, \
             tc.tile_pool(name="eqP", bufs=1) as peq, \
             tc.tile_pool(name="vP", bufs=2) as pv, \
             tc.tile_pool(name="hdP", bufs=4) as phd, \
             tc.tile_pool(name="owP", bufs=2) as pow_, \
             tc.tile_pool(name="gC", bufs=2) as pg, \
             tc.tile_pool(name="agC", bufs=1) as pag, \
             tc.tile_pool(name="yC", bufs=2) as py, \
             tc.tile_pool(name="hC", bufs=1) as ph, \
             tc.tile_pool(name="gX", bufs=2) as pgx, \
             tc.tile_pool(name="psB", bufs=6, space="PSUM") as psb, \
             tc.tile_pool(name="ptB", bufs=2, space="PSUM") as ptb:

            # ---- persistent weights / consts ----
            w1_sb = pcw.tile([128, 6, 4 * DIM], F8)
            nc.sync.dma_start(out=w1_sb[:], in_=w1_d.rearrange("(k p) n -> p k n", p=128))
            b1_sb = pcw.tile([128, 24], F32)
            nc.sync.dma_start(out=b1_sb[:], in_=b1_d.rearrange("a p -> p a"))
            if not _CACHE.get('b2_zero'):
                b2_sb = pcw.tile([128, DIM], F32)
                nc.gpsimd.dma_start(out=b2_sb[:], in_=b2_d[0:1, :].to_broadcast((128, DIM)))
            w2_sb = pcw.tile([128, 24, DIM], BF16)
            nc.sync.dma_start(out=w2_sb[:], in_=w2_d.rearrange("(k p) n -> p k n", p=128))
            eps_t = pcw.tile([128, 1], F32)
            nc.vector.memset(eps_t[:], EPS)
            ident = pcw.tile([128, 128], F8)
            make_identity(nc, ident[:])
            wqk_sb = pcw.tile([128, 6, 2 * DIM], F8)
            nc.sync.dma_start(out=wqk_sb[:], in_=wqk_d.rearrange("(k p) n -> p k n", p=128))
            wv_sb = pcw.tile([128, 6, DIM], F8)
            nc.sync.dma_start(out=wv_sb[:], in_=wv_d.rearrange("(k p) n -> p k n", p=128))
            wp_sb = pcw.tile([128, 6, DIM], F8)
            nc.sync.dma_start(out=wp_sb[:], in_=wp_d.rearrange("(k p) n -> p k n", p=128))
            bqk_sb = pcw.tile([128, 12], F32)
            nc.sync.dma_start(out=bqk_sb[:], in_=bqk_d.rearrange("a p -> p a"))
            if not _CACHE.get('vb_zero'):
                vb_sb = pcw.tile([128, DIM], F32)
                nc.gpsimd.dma_start(out=vb_sb[:], in_=vb_d[0:1, :].to_broadcast((128, DIM)))
            if not _CACHE.get('pb_zero'):
                pb_sb = pcw.tile([128, DIM], F32)
                nc.gpsimd.dma_start(out=pb_sb[:], in_=pb_d[0:1, :].to_broadcast((128, DIM)))

            sig_gelu = os.environ.get('KERNEL_GELU') == 'sig'

            def emit_ln1_band(band):
                """LN1 for this band's token tiles; batched sqrt for the band."""
                nbt = band_tiles[band]
                xts = []
                mvb = pa.tile([128, 2, 7], F32, tag="mvb")
                for bt in range(nbt):
                    t = band * 7 + bt
                    xt = pxt.tile([128, DIM], F32, tag="xt")
                    nc.sync.dma_start(out=xt[:], in_=x_t32[t])
                    st = pa.tile([128, 2, 6], F32, tag="st")
                    for s in range(2):
                        nc.vector.bn_stats(out=st[:, s, :], in_=xt[:, s * 384:(s + 1) * 384])
                    nc.vector.bn_aggr(out=mvb[:, :, bt], in_=st[:])
                    xts.append(xt)
                sdb = pa.tile([128, 7], F32, tag="sdb")
                nc.scalar.activation(out=sdb[:, 0:nbt], in_=mvb[:, 1, 0:nbt],
                                     func=mybir.ActivationFunctionType.Sqrt,
                                     bias=eps_t[:], scale=1.0)
                rsd = pa.tile([128, 7], F32, tag="rsd")
                nc.vector.reciprocal(out=rsd[:, 0:nbt], in_=sdb[:, 0:nbt])
                for bt in range(nbt):
                    xn = pa.tile([128, DIM], F8, tag="xn")
                    nc.vector.tensor_scalar(out=xn[:], in0=xts[bt][:],
                                            scalar1=mvb[:, 0, bt:bt + 1],
                                            scalar2=rsd[:, bt:bt + 1],
                                            op0=mybir.AluOpType.subtract,
                                            op1=mybir.AluOpType.mult)
                    nc.sync.dma_start(out=xn1b_t[band][bt], in_=xn[:])

            def emit_window_group(wins):
                """One group (pair or lone window): qkv, per-window V + pipelined
                heads + proj + scatter."""
                nwin = len(wins)
                wfree = N * nwin
                FPAD = 400 if nwin == 2 else 208   # fp8 Ko-step must be %16
                xwtb = pxwt.tile([128, 6, FPAD], F8, tag="xwtb")
                qkt = pqk.tile([128, 12, wfree], F8, tag="qkt")
                att = pxwt.tile([128, 6, FPAD], F8, tag="att")

                # gather + transpose into xwtb
                for ww_i, w in enumerate(wins):
                    woff = ww_i * N
                    wr, wc = _win_rc(w)
                    vr, vc = _valid(w)
                    edge = (vr < 14) or (vc < 14)
                    xw = pxw.tile([128, 2, DIM], F8, tag="xw")
                    if edge:
                        nc.gpsimd.memset(xw[0:CH, 0, :], 0.0)
                        nc.gpsimd.memset(xw[0:CH, 1, :], 0.0)
                        for r in range(vr):
                            c, p0 = r // 7, (r % 7) * WS
                            nc.gpsimd.dma_start(
                                out=xw[p0:p0 + vc, c, :],
                                in_=xn1b_img[wr][r, wc * WS:wc * WS + vc, :])
                    else:
                        for c in range(2):
                            nc.gpsimd.dma_start(
                                out=xw[0:CH, c, :],
                                in_=xn1b_img[wr][c * 7:c * 7 + 7,
                                                 wc * WS:wc * WS + WS, :])
                    # transpose via regular identity matmul: out = xw_slice.T @ I
                    for c, coff in ((0, 0), (1, CH)):
                        for j in range(6):
                            pt = ptb.tile([128, 128], F32, tag="pt")
                            nc.tensor.matmul(
                                pt[0:128, 0:CH],
                                lhsT=xw[0:CH, c, j * 128:(j + 1) * 128],
                                rhs=ident[0:CH, 0:CH],
                                start=True, stop=True)
                            nc.vector.tensor_copy(
                                out=xwtb[:, j, woff + coff:woff + coff + CH],
                                in_=pt[0:128, 0:CH])

                # qkv^T for the whole group (fp8 DoubleRow over k-tile pairs)
                for oc in range(12):
                    pqm = psb.tile([128, 392], F32, tag="ps")
                    for kp in range(3):
                        nc.tensor.matmul(
                            pqm[:, 0:wfree],
                            lhsT=wqk_sb[:, 2 * kp:2 * kp + 2, oc * 128:(oc + 1) * 128],
                            rhs=xwtb[:, 2 * kp:2 * kp + 2, 0:wfree],
                            perf_mode=DR,
                            start=(kp == 0), stop=(kp == 2))
                    nc.vector.tensor_scalar(out=qkt[:, oc, :], in0=pqm[:, 0:wfree],
                                            scalar1=inv_w,
                                            scalar2=bqk_sb[:, oc:oc + 1],
                                            op0=mybir.AluOpType.mult,
                                            op1=mybir.AluOpType.add)

                for ww_i, w in enumerate(wins):
                    woff = ww_i * N
                    # V (fp8): all heads + 64 ones columns for the denominator
                    va = pv.tile([128, 2, DIM + 64], F8, tag="va")
                    for c, coff in ((0, 0), (1, CH)):
                        nc.gpsimd.memset(va[0:CH, c, DIM:DIM + 64], 1.0)
                        pv0 = psb.tile([128, 384], F32, tag="ps")
                        pv1 = psb.tile([128, 384], F32, tag="ps")
                        for kp in range(3):
                            nc.tensor.matmul(
                                pv0[0:CH, :],
                                lhsT=xwtb[:, 2 * kp:2 * kp + 2,
                                          woff + coff:woff + coff + CH],
                                rhs=wv_sb[:, 2 * kp:2 * kp + 2, 0:384],
                                perf_mode=DR,
                                start=(kp == 0), stop=(kp == 2))
                            nc.tensor.matmul(
                                pv1[0:CH, :],
                                lhsT=xwtb[:, 2 * kp:2 * kp + 2,
                                          woff + coff:woff + coff + CH],
                                rhs=wv_sb[:, 2 * kp:2 * kp + 2, 384:768],
                                perf_mode=DR,
                                start=(kp == 0), stop=(kp == 2))
                        for half, pvm in ((0, pv0), (1, pv1)):
                            if _CACHE.get('vb_zero'):
                                nc.vector.tensor_scalar(
                                    out=va[0:CH, c, half * 384:(half + 1) * 384],
                                    in0=pvm[0:CH, :], scalar1=inv_w, scalar2=None,
                                    op0=mybir.AluOpType.mult)
                            else:
                                nc.vector.scalar_tensor_tensor(
                                    out=va[0:CH, c, half * 384:(half + 1) * 384],
                                    in0=pvm[0:CH, :],
                                    scalar=inv_w,
                                    in1=vb_sb[0:CH, half * 384:(half + 1) * 384],
                                    op0=mybir.AluOpType.mult,
                                    op1=mybir.AluOpType.add)

                    eqt = peq.tile([28, NH, N], F8, tag="eqt")
                    nc.sync.dma_start(out=eqt[:], in_=eq_d[w].rearrange("h r i -> r h i"))
                    ekt = peq.tile([28, NH, N], F8, tag="ekt")
                    nc.sync.dma_start(out=ekt[:], in_=ek_d[w].rearrange("h r i -> r h i"))

                    # heads: QK/bias+exp for pair p, then PV/normalize for pair p-1
                    pTs = {}
                    psos = {}

                    def emit_qk(h):
                        hp = (h % 2) * 64
                        qT = qkt[hp:hp + 64, h // 2, woff:woff + N]
                        kT = qkt[hp:hp + 64, 6 + h // 2, woff:woff + N]
                        pss = psb.tile([128, 2 * N], F32, tag="ps")
                        for c in range(2):
                            nc.tensor.matmul(pss[0:CH, c * N:(c + 1) * N],
                                             lhsT=kT[:, c * CH:(c + 1) * CH], rhs=qT,
                                             start=True, stop=False)
                            nc.tensor.matmul(pss[0:CH, c * N:(c + 1) * N],
                                             lhsT=ekt[:, h, c * CH:(c + 1) * CH],
                                             rhs=eqt[:, h, :],
                                             start=False, stop=True)
                        pT = phd.tile([128, 2, 208], F8, tag="pT")
                        nc.scalar.activation(out=pT[0:CH, :, 0:N], in_=pss[0:CH, 0:2 * N],
                                             func=mybir.ActivationFunctionType.Exp)
                        pTs[h] = pT

                    def emit_pv(p):
                        pso = psb.tile([128, 2 * N], F32, tag="ps")
                        for h in (2 * p, 2 * p + 1):
                            b0 = (h % 2) * 64
                            pT = pTs.pop(h)
                            if b0 == 0:
                                # DoubleRow folds both key-chunks into one pass
                                nc.tensor.matmul(pso[0:64, 0:N],
                                                 lhsT=va[0:CH, 0:2, h * 64:(h + 1) * 64],
                                                 rhs=pT[0:CH, 0:2, 0:N],
                                                 perf_mode=DR, start=True, stop=True,
                                                 skip_group_check=True)
                                nc.tensor.matmul(pso[0:64, N:2 * N],
                                                 lhsT=va[0:CH, 0:2, DIM:DIM + 64],
                                                 rhs=pT[0:CH, 0:2, 0:N],
                                                 perf_mode=DR, start=True, stop=True,
                                                 skip_group_check=True)
                            else:
                                # walrus rejects DoubleRow + col-offset
                                # tile_position; plain fp8 per chunk instead
                                for c in range(2):
                                    nc.tensor.matmul(pso[64:128, 0:N],
                                                     lhsT=va[0:CH, c, h * 64:(h + 1) * 64],
                                                     rhs=pT[0:CH, c, 0:N],
                                                     start=(c == 0), stop=(c == 1),
                                                     skip_group_check=True)
                                    nc.tensor.matmul(pso[64:128, N:2 * N],
                                                     lhsT=va[0:CH, c, DIM:DIM + 64],
                                                     rhs=pT[0:CH, c, 0:N],
                                                     start=(c == 0), stop=(c == 1),
                                                     skip_group_check=True)
                        psos[p] = pso

                    def emit_norm(p):
                        pso = psos.pop(p)
                        rb = phd.tile([128, N], F32, tag="rb")
                        nc.vector.reciprocal(out=rb[:], in_=pso[:, N:2 * N])
                        nc.vector.tensor_mul(out=att[:, p, woff:woff + N],
                                             in0=pso[:, 0:N], in1=rb[:])

                    for p in range(6):
                        emit_qk(2 * p)
                        emit_qk(2 * p + 1)
                        if p >= 1:
                            emit_pv(p - 1)
                            emit_norm(p - 1)
                    emit_pv(5)
                    emit_norm(5)

                    # proj (fp8 DoubleRow) -> ow, then unpartition scatter
                    ow = pow_.tile([128, 2, DIM], BF16, tag="ow")
                    for c, coff in ((0, 0), (1, CH)):
                        pp0 = psb.tile([128, 384], F32, tag="ps")
                        pp1 = psb.tile([128, 384], F32, tag="ps")
                        for kp in range(3):
                            nc.tensor.matmul(
                                pp0[0:CH, :],
                                lhsT=att[:, 2 * kp:2 * kp + 2,
                                         woff + coff:woff + coff + CH],
                                rhs=wp_sb[:, 2 * kp:2 * kp + 2, 0:384],
                                perf_mode=DR,
                                start=(kp == 0), stop=(kp == 2))
                            nc.tensor.matmul(
                                pp1[0:CH, :],
                                lhsT=att[:, 2 * kp:2 * kp + 2,
                                         woff + coff:woff + coff + CH],
                                rhs=wp_sb[:, 2 * kp:2 * kp + 2, 384:768],
                                perf_mode=DR,
                                start=(kp == 0), stop=(kp == 2))
                        for half, psp in ((0, pp0), (1, pp1)):
                            if _CACHE.get('pb_zero'):
                                nc.scalar.activation(
                                    out=ow[0:CH, c, half * 384:(half + 1) * 384],
                                    in_=psp[0:CH, :],
                                    func=mybir.ActivationFunctionType.Copy,
                                    bias=0.0, scale=inv_w)
                            else:
                                nc.vector.scalar_tensor_tensor(
                                    out=ow[0:CH, c, half * 384:(half + 1) * 384],
                                    in0=psp[0:CH, :], scalar=inv_w,
                                    in1=pb_sb[0:CH, half * 384:(half + 1) * 384],
                                    op0=mybir.AluOpType.mult,
                                    op1=mybir.AluOpType.add)
                    wr, wc = _win_rc(w)
                    vr, vc = _valid(w)
                    if vr == 14 and vc == 14:
                        for c in range(2):
                            nc.gpsimd.dma_start(
                                out=at_img[wr * WS + c * 7:wr * WS + c * 7 + 7,
                                           wc * WS:wc * WS + WS, :],
                                in_=ow[0:CH, c, :])
                    else:
                        for r in range(vr):
                            c, p0 = r // 7, (r % 7) * WS
                            nc.gpsimd.dma_start(
                                out=at_img[wr * WS + r, wc * WS:wc * WS + vc, :],
                                in_=ow[p0:p0 + vc, c, :])

            def emit_c_group(g):
                """Phase C for token tiles 2g..2g+1: residual, LN2, MLP, out."""
                xg = pg.tile([128, 2, DIM], F32, tag="xg")
                ag = pag.tile([128, 2, DIM], BF16, tag="ag")
                nc.sync.dma_start(out=xg[:], in_=x_pt[:, 2 * g:2 * g + 2, :])
                nc.sync.dma_start(out=ag[:], in_=at_pt[:, 2 * g:2 * g + 2, :])
                # x2 = x + attn (in place into xg)
                nc.vector.tensor_add(out=xg[:, :, :], in0=xg[:, :, :], in1=ag[:, :, :])
                xn2t = pgx.tile([128, 6, 256], F8, tag="xn2t")
                mvc = pg.tile([128, 2, 2], F32, tag="mvc")
                for s in range(2):
                    st = pg.tile([128, 2, 6], F32, tag="stC")
                    for sub in range(2):
                        nc.vector.bn_stats(out=st[:, sub, :],
                                           in_=xg[:, s, sub * 384:(sub + 1) * 384])
                    nc.vector.bn_aggr(out=mvc[:, :, s], in_=st[:])
                sdc = pg.tile([128, 2], F32, tag="sdC")
                nc.scalar.activation(out=sdc[:], in_=mvc[:, 1, :],
                                     func=mybir.ActivationFunctionType.Sqrt,
                                     bias=eps_t[:], scale=1.0)
                rsc = pg.tile([128, 2], F32, tag="rsC")
                nc.vector.reciprocal(out=rsc[:], in_=sdc[:])
                for s in range(2):
                    xn2b = pg.tile([128, DIM], F8, tag="xn2b")
                    nc.vector.tensor_scalar(out=xn2b[:, :], in0=xg[:, s, :],
                                            scalar1=mvc[:, 0, s:s + 1],
                                            scalar2=rsc[:, s:s + 1],
                                            op0=mybir.AluOpType.subtract,
                                            op1=mybir.AluOpType.mult)
                    if not _CACHE.get('b2_zero'):
                        nc.vector.tensor_add(out=xg[:, s, :], in0=xg[:, s, :],
                                             in1=b2_sb[:])
                    for j in range(6):
                        pt = ptb.tile([128, 128], F32, tag="pt")
                        nc.tensor.matmul(pt[:, :],
                                         lhsT=xn2b[:, j * 128:(j + 1) * 128],
                                         rhs=ident[:, :], start=True, stop=True)
                        nc.vector.tensor_copy(out=xn2t[:, j, s * 128:(s + 1) * 128],
                                              in_=pt[:, :])
                h1t = ph.tile([128, 24, 256], BF16, tag="h1t")
                for oc in range(24):
                    psh = psb.tile([128, 256], F32, tag="ps")
                    for kp in range(3):
                        nc.tensor.matmul(
                            psh[:, :],
                            lhsT=w1_sb[:, 2 * kp:2 * kp + 2, oc * 128:(oc + 1) * 128],
                            rhs=xn2t[:, 2 * kp:2 * kp + 2, :],
                            perf_mode=DR,
                            start=(kp == 0), stop=(kp == 2))
                    if sig_gelu:
                        # CoreSim lacks Gelu; x*sigmoid(1.702x) validates shapes
                        hpre = pg.tile([128, 256], F32, tag="hpre")
                        nc.scalar.activation(out=hpre[:], in_=psh[:, :],
                                             func=mybir.ActivationFunctionType.Identity,
                                             bias=b1_sb[:, oc:oc + 1], scale=inv_w)
                        sg = pg.tile([128, 256], F32, tag="sg")
                        nc.scalar.activation(out=sg[:], in_=hpre[:],
                                             func=mybir.ActivationFunctionType.Sigmoid,
                                             bias=0.0, scale=1.702)
                        nc.vector.tensor_mul(out=h1t[:, oc, :], in0=hpre[:], in1=sg[:])
                    else:
                        nc.scalar.activation(out=h1t[:, oc, :], in_=psh[:, :],
                                             func=mybir.ActivationFunctionType.Gelu,
                                             bias=b1_sb[:, oc:oc + 1], scale=inv_w)
                for s in range(2):
                    pf0 = psb.tile([128, 384], F32, tag="ps")
                    pf1 = psb.tile([128, 384], F32, tag="ps")
                    for kt in range(24):
                        nc.tensor.matmul(
                            pf0[:, :],
                            lhsT=h1t[:, kt, s * 128:(s + 1) * 128],
                            rhs=w2_sb[:, kt, 0:384],
                            start=(kt == 0), stop=(kt == 23))
                        nc.tensor.matmul(
                            pf1[:, :],
                            lhsT=h1t[:, kt, s * 128:(s + 1) * 128],
                            rhs=w2_sb[:, kt, 384:768],
                            start=(kt == 0), stop=(kt == 23))
                    yo = py.tile([128, DIM], F32, tag="yo")
                    for half, psf in ((0, pf0), (1, pf1)):
                        nc.vector.tensor_add(
                            out=yo[:, half * 384:(half + 1) * 384],
                            in0=psf[:, :],
                            in1=xg[:, s, half * 384:(half + 1) * 384])
                    nc.sync.dma_start(out=y_t32[2 * g + s], in_=yo[:])

            # groups of phase C that become ready after each band completes:
            # band b covers image rows 14b..14b+13 -> tokens to (14b+14)*64
            c_ready = {0: [0, 1, 2], 1: [3, 4, 5, 6], 2: [7, 8, 9],
                       3: [10, 11, 12, 13], 4: [14, 15]}

            emit_ln1_band(0)
            for band in range(5):
                w0 = band * NWS
                emit_window_group((w0, w0 + 1))
                # overlap next band's LN1 (DVE/DMA) with this band's windows
                if band < 4:
                    emit_ln1_band(band + 1)
                emit_window_group((w0 + 2, w0 + 3))
                emit_window_group((w0 + 4,))
                for g in c_ready[band]:
                    emit_c_group(g)

    if os.environ.get('KERNEL_NOLDDEDUP') != '1':
        _dedup_ldweights(nc)
    if os.environ.get('KERNEL_SIM') != '1':
        _split_waits(nc)
    _CACHE['nc'] = nc
    return nc


def _host_prep(inputs):
    """Fold LN affines into matmul weights, build rel-pos operands."""
    f32 = np.float32
    x = np.asarray(inputs['x'], f32)
    q_idx = np.asarray(inputs['q_idx']).astype(np.int64)
    k_idx = np.asarray(inputs['k_idx']).astype(np.int64)
    ln1_w = np.asarray(inputs['ln1_w'], f32); ln1_b = np.asarray(inputs['ln1_b'], f32)
    ln2_w = np.asarray(inputs['ln2_w'], f32); ln2_b = np.asarray(inputs['ln2_b'], f32)
    qkv_w = np.asarray(inputs['qkv_w'], f32); qkv_b = np.asarray(inputs['qkv_b'], f32)
    proj_w = np.asarray(inputs['proj_w'], f32); proj_b = np.asarray(inputs['proj_b'], f32)
    mlp_w1 = np.asarray(inputs['mlp_w1'], f32); mlp_b1 = np.asarray(inputs['mlp_b1'], f32)
    mlp_w2 = np.asarray(inputs['mlp_w2'], f32); mlp_b2 = np.asarray(inputs['mlp_b2'], f32)
    rel_h = np.asarray(inputs['rel_h'], f32); rel_w = np.asarray(inputs['rel_w'], f32)

    scale = HD ** -0.5
    Wqkv = ln1_w[:, None] * qkv_w
    bqkv = ln1_b @ qkv_w + qkv_b
    Wqkv = Wqkv.copy(); bqkv = bqkv.copy()
    Wqkv[:, :DIM] *= scale
    bqkv[:DIM] *= scale
    W1 = ln2_w[:, None] * mlp_w1
    b1 = ln2_b @ mlp_w1 + mlp_b1

    coords = np.arange(WS)[:, None] - np.arange(WS)[None, :] + (WS - 1)
    Sh = rel_h[coords].sum(-1).astype(f32)
    Sw = rel_w[coords].sum(-1).astype(f32)

    qr, qc = q_idx // WS, q_idx % WS
    kr, kc = k_idx // WS, k_idx % WS
    nb = q_idx.shape[0]
    Eq = np.concatenate([np.take(Sh, qr, axis=0).transpose(0, 2, 1),
                         np.take(Sw, qc, axis=0).transpose(0, 2, 1)], axis=1)
    Ek = np.zeros((nb, 28, N), f32)
    bi = np.arange(nb)[:, None]
    ar = np.arange(N)[None, :]
    Ek[bi, kr, ar] = 1.0
    Ek[bi, WS + kc, ar] = 1.0

    bf = ml_dtypes.bfloat16
    f8 = ml_dtypes.float8_e4m3fn
    shared = {
        "wqk": np.ascontiguousarray(Wqkv[:, :2 * DIM] * WSCALE).astype(f8),
        "wv": np.ascontiguousarray(Wqkv[:, 2 * DIM:] * WSCALE).astype(f8),
        "bqk": np.ascontiguousarray(bqkv[:2 * DIM].reshape(12, 128)),
        "vb": np.ascontiguousarray(bqkv[2 * DIM:].reshape(1, DIM)),
        "wp": np.ascontiguousarray(proj_w * WSCALE).astype(f8),
        "pb": proj_b.reshape(1, DIM).copy(),
        "w1": np.ascontiguousarray(W1 * WSCALE).astype(f8),
        "b1": np.ascontiguousarray(b1.reshape(24, 128)),
        "w2": mlp_w2.astype(bf),
        "b2": mlp_b2.reshape(1, DIM).copy(),
    }
    Eq = Eq.astype(f8).reshape(B, NW, NH, 28, N)
    Ek = Ek.astype(f8).reshape(B, NW, NH, 28, N)
    in_maps = []
    for b in range(B):
        m = dict(shared)
        m["x"] = np.ascontiguousarray(x[b].reshape(NTOK, DIM))
        m["eq"] = np.ascontiguousarray(Eq[b])
        m["ek"] = np.ascontiguousarray(Ek[b])
        in_maps.append(m)
    return in_maps


def kernel(**inputs):
    in_maps = _host_prep(inputs)
    if 'nc' not in _CACHE:
        _CACHE['pb_zero'] = not np.any(np.asarray(in_maps[0]['pb'], np.float32))
        _CACHE['b2_zero'] = not np.any(np.asarray(in_maps[0]['b2'], np.float32))
    nc = _build()
    trace = os.environ.get('KERNEL_TRACE') == '1'
    if trace:
        _install_ntff_hook()
    res = run_bass_kernel_spmd(nc, in_maps, list(range(B)), trace=trace)
    if trace and res.exec_time_ns is not None:
        print(f"HW exec time: {res.exec_time_ns} ns")
        _CACHE['exec_time_ns'] = res.exec_time_ns
    _CACHE['last_results'] = res
    out = np.stack([np.asarray(res.results[b]["y"]).reshape(HH, WW, DIM)
                    for b in range(B)])
    return out.astype(np.float32)
